# revision 23
# baseline (speedup 1.0000x reference)
# Deformable-conv (DCNv2-style, scrambled-reshape variant) Trainium2 Bass kernel.
# Data-parallel over batch: 8 samples -> 8 NeuronCores.
#
# Per-core pipeline (all layouts derived + validated against the reference in numpy):
#   1. offset conv (18ch, fp16) over padded x -> PE-transpose -> per-n2 selection
#      matmuls -> flat 2x2-patch index f00 + bilinear fracs; gathers can start as
#      soon as idxt[n2] lands. Modulation conv (9ch) over padded x^T + scale
#      table build run behind the first gathers.
#   2. 16 indirect-DMA gathers per (sp, n2) from a host-built patch table
#      (row f = [128 ch x 4 corners] of flat pixels [f, f+1, f+64, f+65], fp16,
#      corner-minor so the scale multiply runs at 2x DVE rate).
#   3. Combine: 2 half-tile muls by (modulation x bilinear) scales, 2 half-tile
#      corner-pair adds (all packed fp16, 2x DVE), one strided final add.
#   4. PE-transpose back to channel-major, Act drains PSUM into vc.
#   5. Main conv = 9 accumulated fp16 matmuls per output tile; Act PSUM copies
#      write through a transposed AP to undo the pi2' ordering.
import sys

import numpy as np

sys.path.insert(0, "/opt/trn_rl_repo")

import concourse.bass as bass
import concourse.bacc as bacc
import concourse.mybir as mybir
from concourse import tile
from concourse.bass_utils import run_bass_kernel_spmd

F32 = mybir.dt.float32
F16 = mybir.dt.float16
I32 = mybir.dt.int32

B, C, H, W = 8, 128, 64, 64
OUT = 256
PIX = H * W            # 4096
KCH = 32               # pixel-major chunks (4096 / 128)
TROWS = 4224           # patch table rows (4096 + pad for f+65 reads)

_CACHE = {}


def _build_host_constants():
    if "sel" in _CACHE:
        return _CACHE
    p2 = np.arange(128)
    k2 = np.arange(KCH)
    sel = np.zeros((9, 3, 128, 128), np.float16)   # [n2, r, p_src, p2]
    basey = np.zeros((9, 128, KCH), np.float32)
    basex = np.zeros((9, 128, KCH), np.float32)
    for n2 in range(9):
        a2, e2 = n2 // 3, n2 % 3
        i2 = p2 % 64
        r = (i2 + e2) % 3
        n = 3 * r + a2                       # source kernel point per partition
        J = (64 * e2 + i2) // 3              # source col j per partition
        c_src = 64 * (p2 // 64) + J          # source partition in pixel-major
        for rr in range(3):
            m = r == rr
            sel[n2, rr, c_src[m], p2[m]] = 1.0
        a = n // 3
        e = n % 3
        # y_u = i + a + o_y ; i = j2 = 2*k2 + p2//64
        basey[n2] = (2 * k2[None, :] + (p2 // 64)[:, None]) + a[:, None]
        basex[n2] = (J + e)[:, None] * np.ones((1, KCH), np.float32)
    _CACHE["sel"] = np.ascontiguousarray(
        sel.transpose(2, 0, 1, 3)).reshape(128, 9 * 3 * 128)
    _CACHE["basey"] = np.ascontiguousarray(
        basey.transpose(1, 0, 2)).reshape(128, 9 * KCH)
    _CACHE["basex"] = np.ascontiguousarray(
        basex.transpose(1, 0, 2)).reshape(128, 9 * KCH)
    _CACHE["ident16"] = np.eye(128, dtype=np.float16)
    return _CACHE


def _pad66(img):  # [C,64,64] -> [C, 66*66] zero-padded
    p = np.zeros((C, 66, 66), np.float16)
    p[:, 1:65, 1:65] = img
    return p.reshape(C, 66 * 66)


def _patch_table(img):  # [C,64,64] f32 -> [TROWS, 512] fp16, rows [ch, corner]
    flat = np.zeros((C, TROWS + 65), np.float16)
    flat[:, :PIX] = img.reshape(C, PIX).astype(np.float16)
    f = np.arange(TROWS)
    tab = np.stack(
        [flat[:, f], flat[:, f + 1], flat[:, f + 64], flat[:, f + 65]], axis=2
    )  # [C, TROWS, 4]
    return np.ascontiguousarray(tab.transpose(1, 0, 2)).reshape(TROWS, 512)


def _build_program():
    if "nc" in _CACHE:
        return _CACHE["nc"]
    nc = bacc.Bacc()
    d = {}
    d["xpad"] = nc.dram_tensor("xpad", [C, 66 * 66], F16, kind="ExternalInput")
    d["xtpad"] = nc.dram_tensor("xtpad", [C, 66 * 66], F16, kind="ExternalInput")
    d["ptab"] = nc.dram_tensor("ptab", [TROWS, 512], F16, kind="ExternalInput")
    d["wom"] = nc.dram_tensor("wom", [C, 9 * 18], F16, kind="ExternalInput")
    d["wmt"] = nc.dram_tensor("wmt", [C, 9 * 9], F16, kind="ExternalInput")
    d["ob"] = nc.dram_tensor("ob", [18, 1], F32, kind="ExternalInput")
    d["mb"] = nc.dram_tensor("mb", [9, 1], F32, kind="ExternalInput")
    d["sel"] = nc.dram_tensor("sel", [128, 9 * 3 * 128], F16, kind="ExternalInput")
    d["basey"] = nc.dram_tensor("basey", [128, 9 * KCH], F32, kind="ExternalInput")
    d["basex"] = nc.dram_tensor("basex", [128, 9 * KCH], F32, kind="ExternalInput")
    d["w2"] = nc.dram_tensor("w2", [C, 9 * 2 * 128], F16, kind="ExternalInput")
    d["id16"] = nc.dram_tensor("id16", [128, 128], F16, kind="ExternalInput")
    d["out"] = nc.dram_tensor("out", [OUT, PIX], F16, kind="ExternalOutput")

    AO = mybir.AluOpType

    with tile.TileContext(nc) as tc:
        with (
            tc.tile_pool(name="imgs", bufs=1) as imgs,
            tc.tile_pool(name="wts", bufs=1) as wts,
            tc.tile_pool(name="meta", bufs=1) as meta,
            tc.tile_pool(name="big", bufs=2) as big,
            tc.tile_pool(name="ps", bufs=2, space="PSUM") as psp,
            tc.tile_pool(name="pst", bufs=2, space="PSUM") as pst,
            tc.tile_pool(name="gbuf", bufs=2) as gbuf,
            tc.tile_pool(name="vbuf", bufs=2) as vbuf,
            tc.tile_pool(name="obuf", bufs=2) as obuf,
        ):
            # ---- loads: conv1/idx path first, conv2/scale + main-conv later
            xpad = imgs.tile([C, 66 * 66], F16)
            xtpad = imgs.tile([C, 66 * 66], F16)
            wom = wts.tile([C, 9, 18], F16)
            wmt = wts.tile([C, 9, 9], F16)
            ob = wts.tile([18, 1], F32)
            mb = wts.tile([9, 1], F32)
            id16 = wts.tile([128, 128], F16)
            selt = wts.tile([128, 9, 3, 128], F16)
            basey = wts.tile([128, 9, KCH], F32)
            basex = wts.tile([128, 9, KCH], F32)
            w2 = wts.tile([C, 9, 2, 128], F16)
            nc.sync.dma_start(xpad[:], d["xpad"][:])
            nc.sync.dma_start(wom[:], d["wom"][:])
            nc.sync.dma_start(ob[:], d["ob"][:])
            nc.sync.dma_start(id16[:], d["id16"][:])
            nc.sync.dma_start(selt[:], d["sel"][:])
            nc.sync.dma_start(basey[:], d["basey"][:])
            nc.sync.dma_start(basex[:], d["basex"][:])
            nc.sync.dma_start(wmt[:], d["wmt"][:])
            nc.sync.dma_start(mb[:], d["mb"][:])
            nc.sync.dma_start(xtpad[:], d["xtpad"][:])
            nc.sync.dma_start(w2[:], d["w2"][:])

            # ---- PE p-state warmup on the identity while xpad streams in
            for _ in range(24):
                wpt = pst.tile([128, 128], F16, tag="tv", name="wpt")
                nc.tensor.transpose(wpt[:], id16[:], id16[:])

            # ---- conv1 (offsets, 18ch over xpad), transposes interleaved
            ocm = big.tile([128, PIX], F16, tag="big")
            opm = meta.tile([128, KCH, 18], F16)   # pi = 128k+p
            for tl in range(8):
                po = psp.tile([18, 512], F32, tag="mm")
                for t in range(9):
                    dy, dx = t // 3, t % 3
                    off = dy * 66 + dx + tl * 8 * 66
                    rhs1 = bass.AP(
                        tensor=xpad[:].tensor, offset=xpad[:].offset + off,
                        ap=[list(xpad[:].ap[0]), [66, 8], [1, 64]],
                    )
                    nc.tensor.matmul(po[:], wom[:, t, :], rhs1,
                                     start=(t == 0), stop=(t == 8))
                nc.scalar.activation(ocm[0:18, tl * 512:(tl + 1) * 512], po[:],
                                     mybir.ActivationFunctionType.Identity,
                                     bias=ob[:], scale=1.0)
                for k in range(4 * tl, 4 * tl + 4):
                    pt = pst.tile([128, 128], F16, tag="tr")
                    nc.tensor.transpose(pt[:], ocm[:, k * 128:(k + 1) * 128],
                                        id16[:])
                    nc.scalar.copy(opm[:, k, :], pt[:, 0:18])

            # ---- per-n2: selection matmuls -> positions -> idx + fracs.
            # Split by k-half: half 0 covers sp=0's chunks, so its gathers
            # start after only half the pipeline latency.
            idxt = meta.tile([128, 9, KCH], I32)
            Fall = meta.tile([128, 9, KCH, 2], F32)
            KH = KCH // 2
            for hk in range(2):
                for n2 in range(9):
                    oyx = pst.tile([128, KH, 2], F32, tag="oyx")
                    for r in range(3):
                        a2 = n2 // 3
                        ch = 3 * r + a2
                        rhs = bass.AP(
                            tensor=opm[:].tensor,
                            offset=opm[:].offset + ch + hk * KH * 18,
                            ap=[list(opm[:].ap[0]), [18, KH], [9, 2]],
                        )
                        nc.tensor.matmul(oyx[:], selt[:, n2, r, :], rhs,
                                         start=(r == 0), stop=(r == 2))
                    kr = slice(hk * KH, (hk + 1) * KH)
                    P = meta.tile([128, KH, 2], F32, tag="P")
                    nc.vector.tensor_add(P[:, :, 0], oyx[:, :, 0],
                                         basey[:, n2, kr])
                    nc.vector.tensor_add(P[:, :, 1], oyx[:, :, 1],
                                         basex[:, n2, kr])
                    nc.vector.tensor_scalar_max(P[:], P[:], 0.0)
                    nc.vector.tensor_scalar_min(P[:], P[:], 63.0)
                    R0 = meta.tile([128, KH, 2], F32, tag="R0")
                    nc.vector.tensor_scalar(R0[:], P[:], -0.5, 12582912.0,
                                            AO.add, AO.add)
                    nc.vector.tensor_scalar_add(R0[:], R0[:], -12582912.0)
                    nc.vector.tensor_sub(Fall[:, n2, kr], P[:], R0[:])
                    f00 = meta.tile([128, KH], F32, tag="f00")
                    nc.vector.scalar_tensor_tensor(
                        f00[:], R0[:, :, 1], 64.0, R0[:, :, 0], AO.mult, AO.add)
                    nc.vector.tensor_copy(idxt[:, n2, kr], f00[:])

            # ---- conv2 (modulation, 9ch over xtpad) + sigmoid
            mcm = big.tile([128, PIX], F16, tag="big")
            for tl in range(8):
                pm = psp.tile([9, 512], F32, tag="mm")
                for t in range(9):
                    dy, dx = t // 3, t % 3
                    off = dy * 66 + dx + tl * 8 * 66
                    rhs2 = bass.AP(
                        tensor=xtpad[:].tensor, offset=xtpad[:].offset + off,
                        ap=[list(xtpad[:].ap[0]), [66, 8], [1, 64]],
                    )
                    nc.tensor.matmul(pm[:], wmt[:, t, :], rhs2,
                                     start=(t == 0), stop=(t == 8))
                nc.scalar.activation(mcm[0:9, tl * 512:(tl + 1) * 512], pm[:],
                                     mybir.ActivationFunctionType.Sigmoid,
                                     bias=mb[:], scale=1.0)

            mpm = meta.tile([128, KCH, 9], F32)    # pi2' = 128k+p
            for k in range(KCH):
                pt2 = pst.tile([128, 128], F16, tag="tr")
                nc.tensor.transpose(pt2[:], mcm[:, k * 128:(k + 1) * 128], id16[:])
                nc.scalar.copy(mpm[:, k, :], pt2[:, 0:9])

            # ---- scales: modulation x bilinear per corner (fp16 table)
            scal = meta.tile([128, 9, KCH, 4], F16)
            for n2 in range(9):
                mrow = mpm[:, :, n2]
                v1 = meta.tile([128, KCH], F32, tag="v1")
                v0 = meta.tile([128, KCH], F32, tag="v0")
                s4 = meta.tile([128, KCH, 4], F32, tag="s4")
                nc.vector.tensor_mul(v1[:], mrow, Fall[:, n2, :, 1])
                nc.vector.tensor_sub(v0[:], mrow, v1[:])
                nc.vector.tensor_mul(s4[:, :, 1], v0[:], Fall[:, n2, :, 0])
                nc.vector.tensor_sub(s4[:, :, 0], v0[:], s4[:, :, 1])
                nc.vector.tensor_mul(s4[:, :, 3], v1[:], Fall[:, n2, :, 0])
                nc.vector.tensor_sub(s4[:, :, 2], v1[:], s4[:, :, 3])
                nc.vector.tensor_copy(scal[:, n2], s4[:])

            # ---- per spatial-half: gather + combine + transpose; then main conv
            for sp in range(2):
                vc = vbuf.tile([C, 9, 16 * 128], F16, tag="vc")
                for n2 in range(9):
                    g = gbuf.tile([128, 16, 128, 4], F16, tag="g")
                    for kk in range(16):
                        k = sp * 16 + kk
                        dst = bass.AP(
                            tensor=g[:].tensor,
                            offset=g[:].offset + kk * 512,
                            ap=[list(g[:].ap[0]), [1, 512]],
                        )
                        nc.gpsimd.indirect_dma_start(
                            out=dst, out_offset=None,
                            in_=d["ptab"][:],
                            in_offset=bass.IndirectOffsetOnAxis(
                                ap=idxt[:, n2, k:k + 1], axis=0),
                        )
                    # per-half combine chains (all packed fp16 at 2x except
                    # the strided final add); h0 completes while h1 gathers
                    tmp = big.tile([128, 16, 128, 2], F16, tag="big")
                    va = gbuf.tile([128, 16, 128], F16, tag="va")
                    for h in range(2):
                        gm = bass.AP(
                            tensor=g[:].tensor,
                            offset=g[:].offset + h * 8 * 512,
                            ap=[list(g[:].ap[0]), [512, 8], [4, 128], [1, 4]],
                        )
                        sc = bass.AP(
                            tensor=scal[:].tensor,
                            offset=scal[:].offset + n2 * (KCH * 4)
                            + (sp * 16 + h * 8) * 4,
                            ap=[list(scal[:].ap[0]), [4, 8], [0, 128], [1, 4]],
                        )
                        nc.vector.tensor_mul(gm, gm, sc)
                        a0 = bass.AP(
                            tensor=g[:].tensor,
                            offset=g[:].offset + h * 8 * 512,
                            ap=[list(g[:].ap[0]), [512, 8], [4, 128], [1, 2]],
                        )
                        a1 = bass.AP(
                            tensor=g[:].tensor,
                            offset=g[:].offset + h * 8 * 512 + 2,
                            ap=[list(g[:].ap[0]), [512, 8], [4, 128], [1, 2]],
                        )
                        to = bass.AP(
                            tensor=tmp[:].tensor,
                            offset=tmp[:].offset + h * 8 * 256,
                            ap=[list(tmp[:].ap[0]), [256, 8], [2, 128], [1, 2]],
                        )
                        nc.vector.tensor_add(to, a0, a1)
                        t0 = bass.AP(
                            tensor=tmp[:].tensor,
                            offset=tmp[:].offset + h * 8 * 256,
                            ap=[list(tmp[:].ap[0]), [256, 8], [2, 128]],
                        )
                        t1 = bass.AP(
                            tensor=tmp[:].tensor,
                            offset=tmp[:].offset + h * 8 * 256 + 1,
                            ap=[list(tmp[:].ap[0]), [256, 8], [2, 128]],
                        )
                        nc.vector.tensor_add(va[:, h * 8:(h + 1) * 8, :], t0, t1)
                        for kk in range(h * 8, h * 8 + 8):
                            ptv = pst.tile([128, 128], F16, tag="tv")
                            nc.tensor.transpose(ptv[:], va[:, kk, :], id16[:])
                            nc.scalar.copy(vc[:, n2, kk * 128:(kk + 1) * 128],
                                           ptv[:])

                # main conv on this spatial half (pi2' in [sp*2048, +2048))
                outsbs = []
                for hf in range(2):
                    outsbs.append(obuf.tile([128, 16 * 128], F16, tag=f"osb{hf}",
                                            name=f"outsb{hf}"))
                for tl in range(4):
                    for hf in range(2):
                        outsb = outsbs[hf]
                        acc = psp.tile([128, 512], F32, tag="mm")
                        for n2 in range(9):
                            nc.tensor.matmul(
                                acc[:], w2[:, n2, hf, :],
                                vc[:, n2, tl * 512:(tl + 1) * 512],
                                start=(n2 == 0), stop=(n2 == 8))
                        # acc covers pi2' = sp*2048 + tl*512 + [0,512):
                        # j2 = (pi2'//64), i2 = pi2'%64 -> dst elem i2*32 + (j2 - 32*sp)
                        dstap = bass.AP(
                            tensor=outsb[:].tensor,
                            offset=outsb[:].offset + 8 * tl,
                            ap=[list(outsb[:].ap[0]), [1, 8], [32, 64]],
                        )
                        nc.scalar.copy(dstap, acc[:])
                # DMA: out[128hf + o, i2, 32sp + j2'] <- outsb[o, i2*32 + j2']
                for hf in range(2):
                    dd = d["out"]
                    dram = bass.AP(
                        tensor=dd[:].tensor,
                        offset=dd[:].offset + hf * 128 * PIX + 32 * sp,
                        ap=[[PIX, 128], [64, 64], [1, 32]],
                    )
                    nc.sync.dma_start(dram, outsbs[hf][:])

    nc.compile()
    _CACHE["nc"] = nc
    return nc


def _host_inputs(b_x, offset_w, offset_b, mod_w, mod_b, conv_w):
    hc = _build_host_constants()
    img = b_x.astype(np.float32)
    imgT = np.ascontiguousarray(img.transpose(0, 2, 1))
    wom = np.zeros((9, C, 18), np.float16)
    wmt = np.zeros((9, C, 9), np.float16)
    for t in range(9):
        dy, dx = t // 3, t % 3
        wom[t] = offset_w[:, :, dy, dx].T
        wmt[3 * dx + dy] = mod_w[:, :, dy, dx].T
    wom = np.ascontiguousarray(wom.transpose(1, 0, 2)).reshape(C, 9 * 18)
    wmt = np.ascontiguousarray(wmt.transpose(1, 0, 2)).reshape(C, 9 * 9)
    w2 = np.zeros((9, 2, C, 128), np.float16)
    for n2 in range(9):
        a2, e2 = n2 // 3, n2 % 3
        for hf in range(2):
            w2[n2, hf] = conv_w[128 * hf:128 * (hf + 1), :, a2, e2].T.astype(
                np.float16)
    w2 = np.ascontiguousarray(w2.transpose(2, 0, 1, 3)).reshape(C, 9 * 2 * 128)
    return {
        "xpad": _pad66(img),
        "xtpad": _pad66(imgT),
        "ptab": _patch_table(img),
        "wom": wom,
        "wmt": wmt,
        "ob": offset_b.reshape(18, 1).astype(np.float32),
        "mb": mod_b.reshape(9, 1).astype(np.float32),
        "sel": hc["sel"],
        "basey": hc["basey"],
        "basex": hc["basex"],
        "w2": w2,
        "id16": hc["ident16"],
    }


def kernel(x, offset_w, offset_b, mod_w, mod_b, conv_w):
    nc = _build_program()
    in_maps = [
        _host_inputs(x[b], offset_w, offset_b, mod_w, mod_b, conv_w)
        for b in range(B)
    ]
    res = run_bass_kernel_spmd(nc, in_maps, core_ids=list(range(B)))
    out = np.stack([res.results[b]["out"].reshape(OUT, H, W) for b in range(B)])
    return out.astype(np.float32)


if __name__ == "__main__":
    rng = np.random.default_rng(0)
    ins = {
        "x": rng.standard_normal((B, C, H, W), dtype=np.float32),
        "offset_w": (rng.standard_normal((18, C, 3, 3)) / 34).astype(np.float32),
        "offset_b": (rng.standard_normal(18) * 0.01).astype(np.float32),
        "mod_w": (rng.standard_normal((9, C, 3, 3)) / 34).astype(np.float32),
        "mod_b": (rng.standard_normal(9) * 0.01).astype(np.float32),
        "conv_w": (rng.standard_normal((OUT, C, 3, 3)) / 34).astype(np.float32),
    }
    o = kernel(**ins)
    print("out", o.shape, o.dtype, np.abs(o).max())


# revision 27
# speedup vs baseline: 1.0126x; 1.0126x over previous
# Deformable-conv (DCNv2-style, scrambled-reshape variant) Trainium2 Bass kernel.
# Data-parallel over batch: 8 samples -> 8 NeuronCores.
#
# Per-core pipeline (all layouts derived + validated against the reference in numpy):
#   1. offset conv (18ch, fp16) over padded x -> PE-transpose -> per-n2 selection
#      matmuls -> flat 2x2-patch index f00 + bilinear fracs; gathers can start as
#      soon as idxt[n2] lands. Modulation conv (9ch) over padded x^T + scale
#      table build run behind the first gathers.
#   2. 16 indirect-DMA gathers per (sp, n2) from a host-built patch table
#      (row f = [128 ch x 4 corners] of flat pixels [f, f+1, f+64, f+65], fp16,
#      corner-minor so the scale multiply runs at 2x DVE rate).
#   3. Combine: 2 half-tile muls by (modulation x bilinear) scales, 2 half-tile
#      corner-pair adds (all packed fp16, 2x DVE), one strided final add.
#   4. PE-transpose back to channel-major, Act drains PSUM into vc.
#   5. Main conv = 9 accumulated fp16 matmuls per output tile; Act PSUM copies
#      write through a transposed AP to undo the pi2' ordering.
import sys

import numpy as np

sys.path.insert(0, "/opt/trn_rl_repo")

import concourse.bass as bass
import concourse.bacc as bacc
import concourse.mybir as mybir
from concourse import tile
from concourse.bass_utils import run_bass_kernel_spmd

F32 = mybir.dt.float32
F16 = mybir.dt.float16
I32 = mybir.dt.int32

B, C, H, W = 8, 128, 64, 64
OUT = 256
PIX = H * W            # 4096
KCH = 32               # pixel-major chunks (4096 / 128)
TROWS = 4224           # patch table rows (4096 + pad for f+65 reads)

_CACHE = {}


def _build_host_constants():
    if "sel" in _CACHE:
        return _CACHE
    p2 = np.arange(128)
    k2 = np.arange(KCH)
    sel = np.zeros((9, 3, 128, 128), np.float16)   # [n2, r, p_src, p2]
    basey = np.zeros((9, 128, KCH), np.float32)
    basex = np.zeros((9, 128, KCH), np.float32)
    for n2 in range(9):
        a2, e2 = n2 // 3, n2 % 3
        i2 = p2 % 64
        r = (i2 + e2) % 3
        n = 3 * r + a2                       # source kernel point per partition
        J = (64 * e2 + i2) // 3              # source col j per partition
        c_src = 64 * (p2 // 64) + J          # source partition in pixel-major
        for rr in range(3):
            m = r == rr
            sel[n2, rr, c_src[m], p2[m]] = 1.0
        a = n // 3
        e = n % 3
        # y_u = i + a + o_y ; i = j2 = 2*k2 + p2//64
        basey[n2] = (2 * k2[None, :] + (p2 // 64)[:, None]) + a[:, None]
        basex[n2] = (J + e)[:, None] * np.ones((1, KCH), np.float32)
    _CACHE["sel"] = np.ascontiguousarray(
        sel.transpose(2, 0, 1, 3)).reshape(128, 9 * 3 * 128)
    _CACHE["basey"] = np.ascontiguousarray(
        basey.transpose(1, 0, 2)).reshape(128, 9 * KCH)
    _CACHE["basex"] = np.ascontiguousarray(
        basex.transpose(1, 0, 2)).reshape(128, 9 * KCH)
    _CACHE["ident16"] = np.eye(128, dtype=np.float16)
    return _CACHE


def _pad66(img):  # [C,64,64] -> [C, 66*66] zero-padded
    p = np.zeros((C, 66, 66), np.float16)
    p[:, 1:65, 1:65] = img
    return p.reshape(C, 66 * 66)


def _patch_table(img):  # [C,64,64] f32 -> [TROWS, 512] fp16, rows [ch, corner]
    flat = np.zeros((C, TROWS + 65), np.float16)
    flat[:, :PIX] = img.reshape(C, PIX).astype(np.float16)
    f = np.arange(TROWS)
    tab = np.stack(
        [flat[:, f], flat[:, f + 1], flat[:, f + 64], flat[:, f + 65]], axis=2
    )  # [C, TROWS, 4]
    return np.ascontiguousarray(tab.transpose(1, 0, 2)).reshape(TROWS, 512)


def _build_program():
    if "nc" in _CACHE:
        return _CACHE["nc"]
    nc = bacc.Bacc()
    d = {}
    d["xpad"] = nc.dram_tensor("xpad", [C, 66 * 66], F16, kind="ExternalInput")
    d["xtpad"] = nc.dram_tensor("xtpad", [C, 66 * 66], F16, kind="ExternalInput")
    d["ptab"] = nc.dram_tensor("ptab", [TROWS, 512], F16, kind="ExternalInput")
    d["wom"] = nc.dram_tensor("wom", [C, 9 * 18], F16, kind="ExternalInput")
    d["wmt"] = nc.dram_tensor("wmt", [C, 9 * 9], F16, kind="ExternalInput")
    d["ob"] = nc.dram_tensor("ob", [18, 1], F32, kind="ExternalInput")
    d["mb"] = nc.dram_tensor("mb", [9, 1], F32, kind="ExternalInput")
    d["sel"] = nc.dram_tensor("sel", [128, 9 * 3 * 128], F16, kind="ExternalInput")
    d["basey"] = nc.dram_tensor("basey", [128, 9 * KCH], F32, kind="ExternalInput")
    d["basex"] = nc.dram_tensor("basex", [128, 9 * KCH], F32, kind="ExternalInput")
    d["w2"] = nc.dram_tensor("w2", [C, 9 * 2 * 128], F16, kind="ExternalInput")
    d["id16"] = nc.dram_tensor("id16", [128, 128], F16, kind="ExternalInput")
    d["out"] = nc.dram_tensor("out", [OUT, PIX], F16, kind="ExternalOutput")

    AO = mybir.AluOpType

    with tile.TileContext(nc) as tc:
        with (
            tc.tile_pool(name="imgs", bufs=1) as imgs,
            tc.tile_pool(name="wts", bufs=1) as wts,
            tc.tile_pool(name="meta", bufs=1) as meta,
            tc.tile_pool(name="big", bufs=2) as big,
            tc.tile_pool(name="ps", bufs=2, space="PSUM") as psp,
            tc.tile_pool(name="pst", bufs=2, space="PSUM") as pst,
            tc.tile_pool(name="gbuf", bufs=2) as gbuf,
            tc.tile_pool(name="vbuf", bufs=2) as vbuf,
            tc.tile_pool(name="obuf", bufs=2) as obuf,
        ):
            # ---- loads: conv1/idx path first, conv2/scale + main-conv later
            xpad = imgs.tile([C, 66 * 66], F16)
            xtpad = imgs.tile([C, 66 * 66], F16)
            wom = wts.tile([C, 9, 18], F16)
            wmt = wts.tile([C, 9, 9], F16)
            ob = wts.tile([18, 1], F32)
            mb = wts.tile([9, 1], F32)
            id16 = wts.tile([128, 128], F16)
            selt = wts.tile([128, 9, 3, 128], F16)
            basey = wts.tile([128, 9, KCH], F32)
            basex = wts.tile([128, 9, KCH], F32)
            w2 = wts.tile([C, 9, 2, 128], F16)
            nc.sync.dma_start(xpad[:], d["xpad"][:])
            nc.sync.dma_start(wom[:], d["wom"][:])
            nc.sync.dma_start(ob[:], d["ob"][:])
            nc.sync.dma_start(id16[:], d["id16"][:])
            nc.sync.dma_start(selt[:], d["sel"][:])
            nc.sync.dma_start(basey[:], d["basey"][:])
            nc.sync.dma_start(basex[:], d["basex"][:])
            nc.sync.dma_start(wmt[:], d["wmt"][:])
            nc.sync.dma_start(mb[:], d["mb"][:])
            nc.sync.dma_start(xtpad[:], d["xtpad"][:])
            nc.sync.dma_start(w2[:], d["w2"][:])

            # ---- PE p-state warmup on the identity while xpad streams in
            for _ in range(24):
                wpt = pst.tile([128, 128], F16, tag="tv", name="wpt")
                nc.tensor.transpose(wpt[:], id16[:], id16[:])

            # ---- conv1 (offsets, 18ch over xpad), transposes interleaved
            ocm = big.tile([128, PIX], F16, tag="big")
            opm = meta.tile([128, KCH, 18], F16)   # pi = 128k+p
            for tl in range(8):
                po = psp.tile([18, 512], F32, tag="mm")
                for t in range(9):
                    dy, dx = t // 3, t % 3
                    off = dy * 66 + dx + tl * 8 * 66
                    rhs1 = bass.AP(
                        tensor=xpad[:].tensor, offset=xpad[:].offset + off,
                        ap=[list(xpad[:].ap[0]), [66, 8], [1, 64]],
                    )
                    nc.tensor.matmul(po[:], wom[:, t, :], rhs1,
                                     start=(t == 0), stop=(t == 8))
                nc.scalar.activation(ocm[0:18, tl * 512:(tl + 1) * 512], po[:],
                                     mybir.ActivationFunctionType.Identity,
                                     bias=ob[:], scale=1.0)
                for k in range(4 * tl, 4 * tl + 4):
                    pt = pst.tile([128, 128], F16, tag="tr")
                    nc.tensor.transpose(pt[:], ocm[:, k * 128:(k + 1) * 128],
                                        id16[:])
                    nc.scalar.copy(opm[:, k, :], pt[:, 0:18])

            # ---- per-n2: selection matmuls -> positions -> idx + fracs.
            # Split by k-half: half 0 covers sp=0's chunks, so its gathers
            # start after only half the pipeline latency.
            idxt = meta.tile([128, 9, KCH], I32)
            sb4 = meta.tile([128, 9, KCH, 4], F32)
            KH = KCH // 2

            def meta_half(hk):
                for n2 in range(9):
                    oyx = pst.tile([128, KH, 2], F32, tag="oyx", name="oyx")
                    for r in range(3):
                        a2 = n2 // 3
                        ch = 3 * r + a2
                        rhs = bass.AP(
                            tensor=opm[:].tensor,
                            offset=opm[:].offset + ch + hk * KH * 18,
                            ap=[list(opm[:].ap[0]), [18, KH], [9, 2]],
                        )
                        nc.tensor.matmul(oyx[:], selt[:, n2, r, :], rhs,
                                         start=(r == 0), stop=(r == 2))
                    kr = slice(hk * KH, (hk + 1) * KH)
                    P = meta.tile([128, KH, 2], F32, tag="P", name="P")
                    nc.vector.tensor_add(P[:, :, 0], oyx[:, :, 0],
                                         basey[:, n2, kr])
                    nc.vector.tensor_add(P[:, :, 1], oyx[:, :, 1],
                                         basex[:, n2, kr])
                    nc.vector.tensor_scalar_max(P[:], P[:], 0.0)
                    nc.vector.tensor_scalar_min(P[:], P[:], 63.0)
                    R0 = meta.tile([128, KH, 2], F32, tag="R0", name="R0")
                    nc.vector.tensor_scalar(R0[:], P[:], -0.5, 12582912.0,
                                            AO.add, AO.add)
                    nc.vector.tensor_scalar_add(R0[:], R0[:], -12582912.0)
                    Fh = meta.tile([128, KH, 2], F32, tag="Fh", name="Fh")
                    nc.vector.tensor_sub(Fh[:], P[:], R0[:])
                    f00 = meta.tile([128, KH], F32, tag="f00", name="f00")
                    nc.vector.scalar_tensor_tensor(
                        f00[:], R0[:, :, 1], 64.0, R0[:, :, 0], AO.mult, AO.add)
                    nc.vector.tensor_copy(idxt[:, n2, kr], f00[:])
                    # bilinear-only corner weights (modulation folded later):
                    # c0=(1-F1)(1-F0) c1=(1-F1)F0 c2=F1(1-F0) c3=F1*F0
                    nc.vector.tensor_scalar(sb4[:, n2, kr, 0], Fh[:, :, 1],
                                            -1.0, 1.0, AO.mult, AO.add)
                    nc.vector.tensor_mul(sb4[:, n2, kr, 1], sb4[:, n2, kr, 0],
                                         Fh[:, :, 0])
                    nc.vector.tensor_sub(sb4[:, n2, kr, 0], sb4[:, n2, kr, 0],
                                         sb4[:, n2, kr, 1])
                    nc.vector.tensor_mul(sb4[:, n2, kr, 3], Fh[:, :, 1],
                                         Fh[:, :, 0])
                    nc.vector.tensor_sub(sb4[:, n2, kr, 2], Fh[:, :, 1],
                                         sb4[:, n2, kr, 3])

            meta_half(0)

            # ---- conv2 (modulation, 9ch over xtpad) + sigmoid; runs while
            # sp=0 gathers stream
            mcm = big.tile([128, PIX], F16, tag="big")
            mpm = meta.tile([128, KCH, 9], F32)    # pi2' = 128k+p
            for tl in range(8):
                pm = psp.tile([9, 512], F32, tag="mm")
                for t in range(9):
                    dy, dx = t // 3, t % 3
                    off = dy * 66 + dx + tl * 8 * 66
                    rhs2 = bass.AP(
                        tensor=xtpad[:].tensor, offset=xtpad[:].offset + off,
                        ap=[list(xtpad[:].ap[0]), [66, 8], [1, 64]],
                    )
                    nc.tensor.matmul(pm[:], wmt[:, t, :], rhs2,
                                     start=(t == 0), stop=(t == 8))
                nc.scalar.activation(mcm[0:9, tl * 512:(tl + 1) * 512], pm[:],
                                     mybir.ActivationFunctionType.Sigmoid,
                                     bias=mb[:], scale=1.0)
                for k in range(4 * tl, 4 * tl + 4):
                    pt2 = pst.tile([128, 128], F16, tag="tr")
                    nc.tensor.transpose(pt2[:], mcm[:, k * 128:(k + 1) * 128],
                                        id16[:])
                    nc.scalar.copy(mpm[:, k, :], pt2[:, 0:9])

            # ---- scales: fold modulation into bilinear weights (tiny
            # broadcast muls; conv2-dependent work kept off the combine path)
            scal = meta.tile([128, 9, KCH, 4], F16)

            def scal_fold(hk):
                kr = slice(hk * KH, (hk + 1) * KH)
                for n2 in range(9):
                    mb4 = bass.AP(
                        tensor=mpm[:].tensor,
                        offset=mpm[:].offset + n2 + hk * KH * 9,
                        ap=[list(mpm[:].ap[0]), [9, KH], [0, 4]],
                    )
                    nc.vector.tensor_mul(scal[:, n2, kr], sb4[:, n2, kr], mb4)

            scal_fold(0)
            meta_half(1)
            scal_fold(1)

            # ---- per spatial-half: gather + combine + transpose; then main conv
            for sp in range(2):
                vc = vbuf.tile([C, 9, 16 * 128], F16, tag="vc")
                for n2 in range(9):
                    g = gbuf.tile([128, 16, 128, 4], F16, tag="g")
                    for kk in range(16):
                        k = sp * 16 + kk
                        dst = bass.AP(
                            tensor=g[:].tensor,
                            offset=g[:].offset + kk * 512,
                            ap=[list(g[:].ap[0]), [1, 512]],
                        )
                        nc.gpsimd.indirect_dma_start(
                            out=dst, out_offset=None,
                            in_=d["ptab"][:],
                            in_offset=bass.IndirectOffsetOnAxis(
                                ap=idxt[:, n2, k:k + 1], axis=0),
                        )
                    # per-half combine chains (all packed fp16 at 2x except
                    # the strided final add); h0 completes while h1 gathers
                    tmp = big.tile([128, 16, 128, 2], F16, tag="big")
                    va = gbuf.tile([128, 16, 128], F16, tag="va")
                    for h in range(2):
                        gm = bass.AP(
                            tensor=g[:].tensor,
                            offset=g[:].offset + h * 8 * 512,
                            ap=[list(g[:].ap[0]), [512, 8], [4, 128], [1, 4]],
                        )
                        sc = bass.AP(
                            tensor=scal[:].tensor,
                            offset=scal[:].offset + n2 * (KCH * 4)
                            + (sp * 16 + h * 8) * 4,
                            ap=[list(scal[:].ap[0]), [4, 8], [0, 128], [1, 4]],
                        )
                        nc.vector.tensor_mul(gm, gm, sc)
                        a0 = bass.AP(
                            tensor=g[:].tensor,
                            offset=g[:].offset + h * 8 * 512,
                            ap=[list(g[:].ap[0]), [512, 8], [4, 128], [1, 2]],
                        )
                        a1 = bass.AP(
                            tensor=g[:].tensor,
                            offset=g[:].offset + h * 8 * 512 + 2,
                            ap=[list(g[:].ap[0]), [512, 8], [4, 128], [1, 2]],
                        )
                        to = bass.AP(
                            tensor=tmp[:].tensor,
                            offset=tmp[:].offset + h * 8 * 256,
                            ap=[list(tmp[:].ap[0]), [256, 8], [2, 128], [1, 2]],
                        )
                        nc.vector.tensor_add(to, a0, a1)
                        t0 = bass.AP(
                            tensor=tmp[:].tensor,
                            offset=tmp[:].offset + h * 8 * 256,
                            ap=[list(tmp[:].ap[0]), [256, 8], [2, 128]],
                        )
                        t1 = bass.AP(
                            tensor=tmp[:].tensor,
                            offset=tmp[:].offset + h * 8 * 256 + 1,
                            ap=[list(tmp[:].ap[0]), [256, 8], [2, 128]],
                        )
                        nc.vector.tensor_add(va[:, h * 8:(h + 1) * 8, :], t0, t1)
                        for kk in range(h * 8, h * 8 + 8):
                            ptv = pst.tile([128, 128], F16, tag="tv")
                            nc.tensor.transpose(ptv[:], va[:, kk, :], id16[:])
                            nc.scalar.copy(vc[:, n2, kk * 128:(kk + 1) * 128],
                                           ptv[:])

                # main conv on this spatial half (pi2' in [sp*2048, +2048))
                outsbs = []
                for hf in range(2):
                    outsbs.append(obuf.tile([128, 16 * 128], F16, tag=f"osb{hf}",
                                            name=f"outsb{hf}"))
                for tl in range(4):
                    for hf in range(2):
                        outsb = outsbs[hf]
                        acc = psp.tile([128, 512], F32, tag="mm")
                        for n2 in range(9):
                            nc.tensor.matmul(
                                acc[:], w2[:, n2, hf, :],
                                vc[:, n2, tl * 512:(tl + 1) * 512],
                                start=(n2 == 0), stop=(n2 == 8))
                        # acc covers pi2' = sp*2048 + tl*512 + [0,512):
                        # j2 = (pi2'//64), i2 = pi2'%64 -> dst elem i2*32 + (j2 - 32*sp)
                        dstap = bass.AP(
                            tensor=outsb[:].tensor,
                            offset=outsb[:].offset + 8 * tl,
                            ap=[list(outsb[:].ap[0]), [1, 8], [32, 64]],
                        )
                        nc.scalar.copy(dstap, acc[:])
                # DMA: out[128hf + o, i2, 32sp + j2'] <- outsb[o, i2*32 + j2']
                for hf in range(2):
                    dd = d["out"]
                    dram = bass.AP(
                        tensor=dd[:].tensor,
                        offset=dd[:].offset + hf * 128 * PIX + 32 * sp,
                        ap=[[PIX, 128], [64, 64], [1, 32]],
                    )
                    nc.sync.dma_start(dram, outsbs[hf][:])

    nc.compile()
    _CACHE["nc"] = nc
    return nc


def _host_inputs(b_x, offset_w, offset_b, mod_w, mod_b, conv_w):
    hc = _build_host_constants()
    img = b_x.astype(np.float32)
    imgT = np.ascontiguousarray(img.transpose(0, 2, 1))
    wom = np.zeros((9, C, 18), np.float16)
    wmt = np.zeros((9, C, 9), np.float16)
    for t in range(9):
        dy, dx = t // 3, t % 3
        wom[t] = offset_w[:, :, dy, dx].T
        wmt[3 * dx + dy] = mod_w[:, :, dy, dx].T
    wom = np.ascontiguousarray(wom.transpose(1, 0, 2)).reshape(C, 9 * 18)
    wmt = np.ascontiguousarray(wmt.transpose(1, 0, 2)).reshape(C, 9 * 9)
    w2 = np.zeros((9, 2, C, 128), np.float16)
    for n2 in range(9):
        a2, e2 = n2 // 3, n2 % 3
        for hf in range(2):
            w2[n2, hf] = conv_w[128 * hf:128 * (hf + 1), :, a2, e2].T.astype(
                np.float16)
    w2 = np.ascontiguousarray(w2.transpose(2, 0, 1, 3)).reshape(C, 9 * 2 * 128)
    return {
        "xpad": _pad66(img),
        "xtpad": _pad66(imgT),
        "ptab": _patch_table(img),
        "wom": wom,
        "wmt": wmt,
        "ob": offset_b.reshape(18, 1).astype(np.float32),
        "mb": mod_b.reshape(9, 1).astype(np.float32),
        "sel": hc["sel"],
        "basey": hc["basey"],
        "basex": hc["basex"],
        "w2": w2,
        "id16": hc["ident16"],
    }


def kernel(x, offset_w, offset_b, mod_w, mod_b, conv_w):
    nc = _build_program()
    in_maps = [
        _host_inputs(x[b], offset_w, offset_b, mod_w, mod_b, conv_w)
        for b in range(B)
    ]
    res = run_bass_kernel_spmd(nc, in_maps, core_ids=list(range(B)))
    out = np.stack([res.results[b]["out"].reshape(OUT, H, W) for b in range(B)])
    return out.astype(np.float32)


if __name__ == "__main__":
    rng = np.random.default_rng(0)
    ins = {
        "x": rng.standard_normal((B, C, H, W), dtype=np.float32),
        "offset_w": (rng.standard_normal((18, C, 3, 3)) / 34).astype(np.float32),
        "offset_b": (rng.standard_normal(18) * 0.01).astype(np.float32),
        "mod_w": (rng.standard_normal((9, C, 3, 3)) / 34).astype(np.float32),
        "mod_b": (rng.standard_normal(9) * 0.01).astype(np.float32),
        "conv_w": (rng.standard_normal((OUT, C, 3, 3)) / 34).astype(np.float32),
    }
    o = kernel(**ins)
    print("out", o.shape, o.dtype, np.abs(o).max())


# revision 28
# speedup vs baseline: 1.0322x; 1.0194x over previous
# Deformable-conv (DCNv2-style, scrambled-reshape variant) Trainium2 Bass kernel.
# Data-parallel over batch: 8 samples -> 8 NeuronCores.
#
# Per-core pipeline (all layouts derived + validated against the reference in numpy):
#   1. offset conv (18ch, fp16) over padded x -> PE-transpose -> per-n2 selection
#      matmuls -> flat 2x2-patch index f00 + bilinear fracs; gathers can start as
#      soon as idxt[n2] lands. Modulation conv (9ch) over padded x^T + scale
#      table build run behind the first gathers.
#   2. 16 indirect-DMA gathers per (sp, n2) from a host-built patch table
#      (row f = [128 ch x 4 corners] of flat pixels [f, f+1, f+64, f+65], fp16,
#      corner-minor so the scale multiply runs at 2x DVE rate).
#   3. Combine: 2 half-tile muls by (modulation x bilinear) scales, 2 half-tile
#      corner-pair adds (all packed fp16, 2x DVE), one strided final add.
#   4. PE-transpose back to channel-major, Act drains PSUM into vc.
#   5. Main conv = 9 accumulated fp16 matmuls per output tile; Act PSUM copies
#      write through a transposed AP to undo the pi2' ordering.
import sys

import numpy as np

sys.path.insert(0, "/opt/trn_rl_repo")

import concourse.bass as bass
import concourse.bacc as bacc
import concourse.mybir as mybir
from concourse import tile
from concourse.bass_utils import run_bass_kernel_spmd

F32 = mybir.dt.float32
F16 = mybir.dt.float16
I32 = mybir.dt.int32

B, C, H, W = 8, 128, 64, 64
OUT = 256
PIX = H * W            # 4096
KCH = 32               # pixel-major chunks (4096 / 128)
TROWS = 4224           # patch table rows (4096 + pad for f+65 reads)

_CACHE = {}


def _build_host_constants():
    if "sel" in _CACHE:
        return _CACHE
    p2 = np.arange(128)
    k2 = np.arange(KCH)
    sel = np.zeros((9, 3, 128, 128), np.float16)   # [n2, r, p_src, p2]
    basey = np.zeros((9, 128, KCH), np.float32)
    basex = np.zeros((9, 128, KCH), np.float32)
    for n2 in range(9):
        a2, e2 = n2 // 3, n2 % 3
        i2 = p2 % 64
        r = (i2 + e2) % 3
        n = 3 * r + a2                       # source kernel point per partition
        J = (64 * e2 + i2) // 3              # source col j per partition
        c_src = 64 * (p2 // 64) + J          # source partition in pixel-major
        for rr in range(3):
            m = r == rr
            sel[n2, rr, c_src[m], p2[m]] = 1.0
        a = n // 3
        e = n % 3
        # y_u = i + a + o_y ; i = j2 = 2*k2 + p2//64
        basey[n2] = (2 * k2[None, :] + (p2 // 64)[:, None]) + a[:, None]
        basex[n2] = (J + e)[:, None] * np.ones((1, KCH), np.float32)
    _CACHE["sel"] = np.ascontiguousarray(
        sel.transpose(2, 0, 1, 3)).reshape(128, 9 * 3 * 128)
    _CACHE["basey"] = np.ascontiguousarray(
        basey.transpose(1, 0, 2)).reshape(128, 9 * KCH)
    _CACHE["basex"] = np.ascontiguousarray(
        basex.transpose(1, 0, 2)).reshape(128, 9 * KCH)
    _CACHE["ident16"] = np.eye(128, dtype=np.float16)
    return _CACHE


def _pad66(img):  # [C,64,64] -> [C, 66*66] zero-padded
    p = np.zeros((C, 66, 66), np.float16)
    p[:, 1:65, 1:65] = img
    return p.reshape(C, 66 * 66)


def _patch_table(img):  # [C,64,64] f32 -> [TROWS, 512] fp16, rows [ch, corner]
    flat = np.zeros((C, TROWS + 65), np.float16)
    flat[:, :PIX] = img.reshape(C, PIX).astype(np.float16)
    f = np.arange(TROWS)
    tab = np.stack(
        [flat[:, f], flat[:, f + 1], flat[:, f + 64], flat[:, f + 65]], axis=2
    )  # [C, TROWS, 4]
    return np.ascontiguousarray(tab.transpose(1, 0, 2)).reshape(TROWS, 512)


def _build_program():
    if "nc" in _CACHE:
        return _CACHE["nc"]
    nc = bacc.Bacc()
    d = {}
    d["xpad"] = nc.dram_tensor("xpad", [C, 66 * 66], F16, kind="ExternalInput")
    d["xtpad"] = nc.dram_tensor("xtpad", [C, 66 * 66], F16, kind="ExternalInput")
    d["ptab"] = nc.dram_tensor("ptab", [TROWS, 512], F16, kind="ExternalInput")
    d["wom"] = nc.dram_tensor("wom", [C, 9 * 18], F16, kind="ExternalInput")
    d["wmt"] = nc.dram_tensor("wmt", [C, 9 * 9], F16, kind="ExternalInput")
    d["ob"] = nc.dram_tensor("ob", [18, 1], F32, kind="ExternalInput")
    d["mb"] = nc.dram_tensor("mb", [9, 1], F32, kind="ExternalInput")
    d["sel"] = nc.dram_tensor("sel", [128, 9 * 3 * 128], F16, kind="ExternalInput")
    d["basey"] = nc.dram_tensor("basey", [128, 9 * KCH], F32, kind="ExternalInput")
    d["basex"] = nc.dram_tensor("basex", [128, 9 * KCH], F32, kind="ExternalInput")
    d["w2"] = nc.dram_tensor("w2", [C, 9 * 2 * 128], F16, kind="ExternalInput")
    d["id16"] = nc.dram_tensor("id16", [128, 128], F16, kind="ExternalInput")
    d["out"] = nc.dram_tensor("out", [OUT, PIX], F16, kind="ExternalOutput")

    AO = mybir.AluOpType

    with tile.TileContext(nc) as tc:
        with (
            tc.tile_pool(name="imgs", bufs=1) as imgs,
            tc.tile_pool(name="wts", bufs=1) as wts,
            tc.tile_pool(name="meta", bufs=1) as meta,
            tc.tile_pool(name="big", bufs=2) as big,
            tc.tile_pool(name="ps", bufs=2, space="PSUM") as psp,
            tc.tile_pool(name="pst", bufs=2, space="PSUM") as pst,
            tc.tile_pool(name="gbuf", bufs=2) as gbuf,
            tc.tile_pool(name="vbuf", bufs=2) as vbuf,
            tc.tile_pool(name="obuf", bufs=2) as obuf,
        ):
            # ---- loads: conv1/idx path first, conv2/scale + main-conv later
            xpad = imgs.tile([C, 66 * 66], F16)
            xtpad = imgs.tile([C, 66 * 66], F16)
            wom = wts.tile([C, 9, 18], F16)
            wmt = wts.tile([C, 9, 9], F16)
            ob = wts.tile([18, 1], F32)
            mb = wts.tile([9, 1], F32)
            id16 = wts.tile([128, 128], F16)
            selt = wts.tile([128, 9, 3, 128], F16)
            basey = wts.tile([128, 9, KCH], F32)
            basex = wts.tile([128, 9, KCH], F32)
            w2 = wts.tile([C, 9, 2, 128], F16)
            nc.sync.dma_start(xpad[:], d["xpad"][:])
            nc.sync.dma_start(wom[:], d["wom"][:])
            nc.sync.dma_start(ob[:], d["ob"][:])
            nc.sync.dma_start(id16[:], d["id16"][:])
            nc.sync.dma_start(selt[:], d["sel"][:])
            nc.sync.dma_start(basey[:], d["basey"][:])
            nc.sync.dma_start(basex[:], d["basex"][:])
            nc.sync.dma_start(wmt[:], d["wmt"][:])
            nc.sync.dma_start(mb[:], d["mb"][:])
            nc.sync.dma_start(xtpad[:], d["xtpad"][:])
            nc.sync.dma_start(w2[:], d["w2"][:])

            # ---- PE p-state warmup on the identity while xpad streams in
            for _ in range(24):
                wpt = pst.tile([128, 128], F16, tag="tv", name="wpt")
                nc.tensor.transpose(wpt[:], id16[:], id16[:])

            # ---- conv1 (offsets, 18ch over xpad), transposes interleaved
            ocm = big.tile([128, PIX], F16, tag="big")
            opm = meta.tile([128, KCH, 18], F16)   # pi = 128k+p

            def conv1_half(hk):
                for tl in range(4 * hk, 4 * hk + 4):
                    po = psp.tile([18, 512], F32, tag="mm", name="po")
                    for t in range(9):
                        dy, dx = t // 3, t % 3
                        off = dy * 66 + dx + tl * 8 * 66
                        rhs1 = bass.AP(
                            tensor=xpad[:].tensor, offset=xpad[:].offset + off,
                            ap=[list(xpad[:].ap[0]), [66, 8], [1, 64]],
                        )
                        nc.tensor.matmul(po[:], wom[:, t, :], rhs1,
                                         start=(t == 0), stop=(t == 8))
                    nc.scalar.activation(ocm[0:18, tl * 512:(tl + 1) * 512],
                                         po[:],
                                         mybir.ActivationFunctionType.Identity,
                                         bias=ob[:], scale=1.0)
                    for k in range(4 * tl, 4 * tl + 4):
                        pt = pst.tile([128, 128], F16, tag="tr", name="pt")
                        nc.tensor.transpose(pt[:], ocm[:, k * 128:(k + 1) * 128],
                                            id16[:])
                        nc.scalar.copy(opm[:, k, :], pt[:, 0:18])

            # ---- per-n2: selection matmuls -> positions -> idx + fracs.
            # Split by k-half: half 0 covers sp=0's chunks, so its gathers
            # start after only half the pipeline latency.
            idxt = meta.tile([128, 9, KCH], I32)
            sb4 = meta.tile([128, 9, KCH, 4], F32)
            KH = KCH // 2

            def meta_half(hk):
                for n2 in range(9):
                    oyx = pst.tile([128, KH, 2], F32, tag="oyx", name="oyx")
                    for r in range(3):
                        a2 = n2 // 3
                        ch = 3 * r + a2
                        rhs = bass.AP(
                            tensor=opm[:].tensor,
                            offset=opm[:].offset + ch + hk * KH * 18,
                            ap=[list(opm[:].ap[0]), [18, KH], [9, 2]],
                        )
                        nc.tensor.matmul(oyx[:], selt[:, n2, r, :], rhs,
                                         start=(r == 0), stop=(r == 2))
                    kr = slice(hk * KH, (hk + 1) * KH)
                    P = meta.tile([128, KH, 2], F32, tag="P", name="P")
                    nc.vector.tensor_add(P[:, :, 0], oyx[:, :, 0],
                                         basey[:, n2, kr])
                    nc.vector.tensor_add(P[:, :, 1], oyx[:, :, 1],
                                         basex[:, n2, kr])
                    nc.vector.tensor_scalar_max(P[:], P[:], 0.0)
                    nc.vector.tensor_scalar_min(P[:], P[:], 63.0)
                    R0 = meta.tile([128, KH, 2], F32, tag="R0", name="R0")
                    nc.vector.tensor_scalar(R0[:], P[:], -0.5, 12582912.0,
                                            AO.add, AO.add)
                    nc.vector.tensor_scalar_add(R0[:], R0[:], -12582912.0)
                    Fh = meta.tile([128, KH, 2], F32, tag="Fh", name="Fh")
                    nc.vector.tensor_sub(Fh[:], P[:], R0[:])
                    f00 = meta.tile([128, KH], F32, tag="f00", name="f00")
                    nc.vector.scalar_tensor_tensor(
                        f00[:], R0[:, :, 1], 64.0, R0[:, :, 0], AO.mult, AO.add)
                    nc.vector.tensor_copy(idxt[:, n2, kr], f00[:])
                    # bilinear-only corner weights (modulation folded later):
                    # c0=(1-F1)(1-F0) c1=(1-F1)F0 c2=F1(1-F0) c3=F1*F0
                    nc.vector.tensor_scalar(sb4[:, n2, kr, 0], Fh[:, :, 1],
                                            -1.0, 1.0, AO.mult, AO.add)
                    nc.vector.tensor_mul(sb4[:, n2, kr, 1], sb4[:, n2, kr, 0],
                                         Fh[:, :, 0])
                    nc.vector.tensor_sub(sb4[:, n2, kr, 0], sb4[:, n2, kr, 0],
                                         sb4[:, n2, kr, 1])
                    nc.vector.tensor_mul(sb4[:, n2, kr, 3], Fh[:, :, 1],
                                         Fh[:, :, 0])
                    nc.vector.tensor_sub(sb4[:, n2, kr, 2], Fh[:, :, 1],
                                         sb4[:, n2, kr, 3])

            # ---- conv2 (modulation, 9ch over xtpad) + sigmoid, per-half
            mcm = big.tile([128, PIX], F16, tag="big")
            mpm = meta.tile([128, KCH, 9], F32)    # pi2' = 128k+p

            def conv2_half(hk):
                for tl in range(4 * hk, 4 * hk + 4):
                    pm = psp.tile([9, 512], F32, tag="mm", name="pm")
                    for t in range(9):
                        dy, dx = t // 3, t % 3
                        off = dy * 66 + dx + tl * 8 * 66
                        rhs2 = bass.AP(
                            tensor=xtpad[:].tensor,
                            offset=xtpad[:].offset + off,
                            ap=[list(xtpad[:].ap[0]), [66, 8], [1, 64]],
                        )
                        nc.tensor.matmul(pm[:], wmt[:, t, :], rhs2,
                                         start=(t == 0), stop=(t == 8))
                    nc.scalar.activation(mcm[0:9, tl * 512:(tl + 1) * 512],
                                         pm[:],
                                         mybir.ActivationFunctionType.Sigmoid,
                                         bias=mb[:], scale=1.0)
                    for k in range(4 * tl, 4 * tl + 4):
                        pt2 = pst.tile([128, 128], F16, tag="tr", name="pt2")
                        nc.tensor.transpose(pt2[:],
                                            mcm[:, k * 128:(k + 1) * 128],
                                            id16[:])
                        nc.scalar.copy(mpm[:, k, :], pt2[:, 0:9])

            # ---- scales: fold modulation into bilinear weights (tiny
            # broadcast muls; conv2-dependent work kept off the combine path)
            scal = meta.tile([128, 9, KCH, 4], F16)

            def scal_fold(hk):
                kr = slice(hk * KH, (hk + 1) * KH)
                for n2 in range(9):
                    mb4 = bass.AP(
                        tensor=mpm[:].tensor,
                        offset=mpm[:].offset + n2 + hk * KH * 9,
                        ap=[list(mpm[:].ap[0]), [9, KH], [0, 4]],
                    )
                    nc.vector.tensor_mul(scal[:, n2, kr], sb4[:, n2, kr], mb4)

            conv1_half(0)
            meta_half(0)
            conv2_half(0)
            scal_fold(0)
            conv1_half(1)
            meta_half(1)
            conv2_half(1)
            scal_fold(1)

            # ---- per spatial-half: gather + combine + transpose; then main conv
            for sp in range(2):
                vc = vbuf.tile([C, 9, 16 * 128], F16, tag="vc")
                for n2 in range(9):
                    g = gbuf.tile([128, 16, 128, 4], F16, tag="g")
                    for kk in range(16):
                        k = sp * 16 + kk
                        dst = bass.AP(
                            tensor=g[:].tensor,
                            offset=g[:].offset + kk * 512,
                            ap=[list(g[:].ap[0]), [1, 512]],
                        )
                        nc.gpsimd.indirect_dma_start(
                            out=dst, out_offset=None,
                            in_=d["ptab"][:],
                            in_offset=bass.IndirectOffsetOnAxis(
                                ap=idxt[:, n2, k:k + 1], axis=0),
                        )
                    # per-half combine chains (all packed fp16 at 2x except
                    # the strided final add); h0 completes while h1 gathers
                    tmp = big.tile([128, 16, 128, 2], F16, tag="big")
                    va = gbuf.tile([128, 16, 128], F16, tag="va")
                    for h in range(2):
                        gm = bass.AP(
                            tensor=g[:].tensor,
                            offset=g[:].offset + h * 8 * 512,
                            ap=[list(g[:].ap[0]), [512, 8], [4, 128], [1, 4]],
                        )
                        sc = bass.AP(
                            tensor=scal[:].tensor,
                            offset=scal[:].offset + n2 * (KCH * 4)
                            + (sp * 16 + h * 8) * 4,
                            ap=[list(scal[:].ap[0]), [4, 8], [0, 128], [1, 4]],
                        )
                        nc.vector.tensor_mul(gm, gm, sc)
                        a0 = bass.AP(
                            tensor=g[:].tensor,
                            offset=g[:].offset + h * 8 * 512,
                            ap=[list(g[:].ap[0]), [512, 8], [4, 128], [1, 2]],
                        )
                        a1 = bass.AP(
                            tensor=g[:].tensor,
                            offset=g[:].offset + h * 8 * 512 + 2,
                            ap=[list(g[:].ap[0]), [512, 8], [4, 128], [1, 2]],
                        )
                        to = bass.AP(
                            tensor=tmp[:].tensor,
                            offset=tmp[:].offset + h * 8 * 256,
                            ap=[list(tmp[:].ap[0]), [256, 8], [2, 128], [1, 2]],
                        )
                        nc.vector.tensor_add(to, a0, a1)
                        t0 = bass.AP(
                            tensor=tmp[:].tensor,
                            offset=tmp[:].offset + h * 8 * 256,
                            ap=[list(tmp[:].ap[0]), [256, 8], [2, 128]],
                        )
                        t1 = bass.AP(
                            tensor=tmp[:].tensor,
                            offset=tmp[:].offset + h * 8 * 256 + 1,
                            ap=[list(tmp[:].ap[0]), [256, 8], [2, 128]],
                        )
                        nc.vector.tensor_add(va[:, h * 8:(h + 1) * 8, :], t0, t1)
                        for kk in range(h * 8, h * 8 + 8):
                            ptv = pst.tile([128, 128], F16, tag="tv")
                            nc.tensor.transpose(ptv[:], va[:, kk, :], id16[:])
                            nc.scalar.copy(vc[:, n2, kk * 128:(kk + 1) * 128],
                                           ptv[:])

                # main conv on this spatial half (pi2' in [sp*2048, +2048))
                outsbs = []
                for hf in range(2):
                    outsbs.append(obuf.tile([128, 16 * 128], F16, tag=f"osb{hf}",
                                            name=f"outsb{hf}"))
                for tl in range(4):
                    for hf in range(2):
                        outsb = outsbs[hf]
                        acc = psp.tile([128, 512], F32, tag="mm")
                        for n2 in range(9):
                            nc.tensor.matmul(
                                acc[:], w2[:, n2, hf, :],
                                vc[:, n2, tl * 512:(tl + 1) * 512],
                                start=(n2 == 0), stop=(n2 == 8))
                        # acc covers pi2' = sp*2048 + tl*512 + [0,512):
                        # j2 = (pi2'//64), i2 = pi2'%64 -> dst elem i2*32 + (j2 - 32*sp)
                        dstap = bass.AP(
                            tensor=outsb[:].tensor,
                            offset=outsb[:].offset + 8 * tl,
                            ap=[list(outsb[:].ap[0]), [1, 8], [32, 64]],
                        )
                        nc.scalar.copy(dstap, acc[:])
                # DMA: out[128hf + o, i2, 32sp + j2'] <- outsb[o, i2*32 + j2']
                for hf in range(2):
                    dd = d["out"]
                    dram = bass.AP(
                        tensor=dd[:].tensor,
                        offset=dd[:].offset + hf * 128 * PIX + 32 * sp,
                        ap=[[PIX, 128], [64, 64], [1, 32]],
                    )
                    nc.sync.dma_start(dram, outsbs[hf][:])

    nc.compile()
    _CACHE["nc"] = nc
    return nc


def _host_inputs(b_x, offset_w, offset_b, mod_w, mod_b, conv_w):
    hc = _build_host_constants()
    img = b_x.astype(np.float32)
    imgT = np.ascontiguousarray(img.transpose(0, 2, 1))
    wom = np.zeros((9, C, 18), np.float16)
    wmt = np.zeros((9, C, 9), np.float16)
    for t in range(9):
        dy, dx = t // 3, t % 3
        wom[t] = offset_w[:, :, dy, dx].T
        wmt[3 * dx + dy] = mod_w[:, :, dy, dx].T
    wom = np.ascontiguousarray(wom.transpose(1, 0, 2)).reshape(C, 9 * 18)
    wmt = np.ascontiguousarray(wmt.transpose(1, 0, 2)).reshape(C, 9 * 9)
    w2 = np.zeros((9, 2, C, 128), np.float16)
    for n2 in range(9):
        a2, e2 = n2 // 3, n2 % 3
        for hf in range(2):
            w2[n2, hf] = conv_w[128 * hf:128 * (hf + 1), :, a2, e2].T.astype(
                np.float16)
    w2 = np.ascontiguousarray(w2.transpose(2, 0, 1, 3)).reshape(C, 9 * 2 * 128)
    return {
        "xpad": _pad66(img),
        "xtpad": _pad66(imgT),
        "ptab": _patch_table(img),
        "wom": wom,
        "wmt": wmt,
        "ob": offset_b.reshape(18, 1).astype(np.float32),
        "mb": mod_b.reshape(9, 1).astype(np.float32),
        "sel": hc["sel"],
        "basey": hc["basey"],
        "basex": hc["basex"],
        "w2": w2,
        "id16": hc["ident16"],
    }


def kernel(x, offset_w, offset_b, mod_w, mod_b, conv_w):
    nc = _build_program()
    in_maps = [
        _host_inputs(x[b], offset_w, offset_b, mod_w, mod_b, conv_w)
        for b in range(B)
    ]
    res = run_bass_kernel_spmd(nc, in_maps, core_ids=list(range(B)))
    out = np.stack([res.results[b]["out"].reshape(OUT, H, W) for b in range(B)])
    return out.astype(np.float32)


if __name__ == "__main__":
    rng = np.random.default_rng(0)
    ins = {
        "x": rng.standard_normal((B, C, H, W), dtype=np.float32),
        "offset_w": (rng.standard_normal((18, C, 3, 3)) / 34).astype(np.float32),
        "offset_b": (rng.standard_normal(18) * 0.01).astype(np.float32),
        "mod_w": (rng.standard_normal((9, C, 3, 3)) / 34).astype(np.float32),
        "mod_b": (rng.standard_normal(9) * 0.01).astype(np.float32),
        "conv_w": (rng.standard_normal((OUT, C, 3, 3)) / 34).astype(np.float32),
    }
    o = kernel(**ins)
    print("out", o.shape, o.dtype, np.abs(o).max())


# revision 29
# speedup vs baseline: 1.0324x; 1.0001x over previous
# Deformable-conv (DCNv2-style, scrambled-reshape variant) Trainium2 Bass kernel.
# Data-parallel over batch: 8 samples -> 8 NeuronCores.
#
# Per-core pipeline (all layouts derived + validated against the reference in numpy):
#   1. offset conv (18ch, fp16) over padded x -> PE-transpose -> per-n2 selection
#      matmuls -> flat 2x2-patch index f00 + bilinear fracs; gathers can start as
#      soon as idxt[n2] lands. Modulation conv (9ch) over padded x^T + scale
#      table build run behind the first gathers.
#   2. 16 indirect-DMA gathers per (sp, n2) from a host-built patch table
#      (row f = [128 ch x 4 corners] of flat pixels [f, f+1, f+64, f+65], fp16,
#      corner-minor so the scale multiply runs at 2x DVE rate).
#   3. Combine: 2 half-tile muls by (modulation x bilinear) scales, 2 half-tile
#      corner-pair adds (all packed fp16, 2x DVE), one strided final add.
#   4. PE-transpose back to channel-major, Act drains PSUM into vc.
#   5. Main conv = 9 accumulated fp16 matmuls per output tile; Act PSUM copies
#      write through a transposed AP to undo the pi2' ordering.
import sys

import numpy as np

sys.path.insert(0, "/opt/trn_rl_repo")

import concourse.bass as bass
import concourse.bacc as bacc
import concourse.mybir as mybir
from concourse import tile
from concourse.bass_utils import run_bass_kernel_spmd

F32 = mybir.dt.float32
F16 = mybir.dt.float16
I32 = mybir.dt.int32

B, C, H, W = 8, 128, 64, 64
OUT = 256
PIX = H * W            # 4096
KCH = 32               # pixel-major chunks (4096 / 128)
TROWS = 4224           # patch table rows (4096 + pad for f+65 reads)

_CACHE = {}


def _build_host_constants():
    if "sel" in _CACHE:
        return _CACHE
    p2 = np.arange(128)
    k2 = np.arange(KCH)
    sel = np.zeros((9, 3, 128, 128), np.float16)   # [n2, r, p_src, p2]
    basey = np.zeros((9, 128, KCH), np.float32)
    basex = np.zeros((9, 128, KCH), np.float32)
    for n2 in range(9):
        a2, e2 = n2 // 3, n2 % 3
        i2 = p2 % 64
        r = (i2 + e2) % 3
        n = 3 * r + a2                       # source kernel point per partition
        J = (64 * e2 + i2) // 3              # source col j per partition
        c_src = 64 * (p2 // 64) + J          # source partition in pixel-major
        for rr in range(3):
            m = r == rr
            sel[n2, rr, c_src[m], p2[m]] = 1.0
        a = n // 3
        e = n % 3
        # y_u = i + a + o_y ; i = j2 = 2*k2 + p2//64
        basey[n2] = (2 * k2[None, :] + (p2 // 64)[:, None]) + a[:, None]
        basex[n2] = (J + e)[:, None] * np.ones((1, KCH), np.float32)
    _CACHE["sel"] = np.ascontiguousarray(
        sel.transpose(2, 0, 1, 3)).reshape(128, 9 * 3 * 128)
    _CACHE["basey"] = np.ascontiguousarray(
        basey.transpose(1, 0, 2)).reshape(128, 9 * KCH)
    _CACHE["basex"] = np.ascontiguousarray(
        basex.transpose(1, 0, 2)).reshape(128, 9 * KCH)
    _CACHE["ident16"] = np.eye(128, dtype=np.float16)
    return _CACHE


def _pad66(img):  # [C,64,64] -> [C, 66*66] zero-padded
    p = np.zeros((C, 66, 66), np.float16)
    p[:, 1:65, 1:65] = img
    return p.reshape(C, 66 * 66)


def _patch_table(img):  # [C,64,64] f32 -> [TROWS, 512] fp16, rows [ch, corner]
    flat = np.zeros((C, TROWS + 65), np.float16)
    flat[:, :PIX] = img.reshape(C, PIX).astype(np.float16)
    f = np.arange(TROWS)
    tab = np.stack(
        [flat[:, f], flat[:, f + 1], flat[:, f + 64], flat[:, f + 65]], axis=2
    )  # [C, TROWS, 4]
    return np.ascontiguousarray(tab.transpose(1, 0, 2)).reshape(TROWS, 512)


def _build_program():
    if "nc" in _CACHE:
        return _CACHE["nc"]
    nc = bacc.Bacc()
    d = {}
    d["xpad"] = nc.dram_tensor("xpad", [C, 66 * 66], F16, kind="ExternalInput")
    d["xtpad"] = nc.dram_tensor("xtpad", [C, 66 * 66], F16, kind="ExternalInput")
    d["ptab"] = nc.dram_tensor("ptab", [TROWS, 512], F16, kind="ExternalInput")
    d["wom"] = nc.dram_tensor("wom", [C, 9 * 18], F16, kind="ExternalInput")
    d["wmt"] = nc.dram_tensor("wmt", [C, 9 * 9], F16, kind="ExternalInput")
    d["ob"] = nc.dram_tensor("ob", [18, 1], F32, kind="ExternalInput")
    d["mb"] = nc.dram_tensor("mb", [9, 1], F32, kind="ExternalInput")
    d["sel"] = nc.dram_tensor("sel", [128, 9 * 3 * 128], F16, kind="ExternalInput")
    d["basey"] = nc.dram_tensor("basey", [128, 9 * KCH], F32, kind="ExternalInput")
    d["basex"] = nc.dram_tensor("basex", [128, 9 * KCH], F32, kind="ExternalInput")
    d["w2"] = nc.dram_tensor("w2", [C, 9 * 2 * 128], F16, kind="ExternalInput")
    d["id16"] = nc.dram_tensor("id16", [128, 128], F16, kind="ExternalInput")
    d["out"] = nc.dram_tensor("out", [OUT, PIX], F16, kind="ExternalOutput")

    AO = mybir.AluOpType

    with tile.TileContext(nc) as tc:
        with (
            tc.tile_pool(name="imgs", bufs=1) as imgs,
            tc.tile_pool(name="wts", bufs=1) as wts,
            tc.tile_pool(name="meta", bufs=1) as meta,
            tc.tile_pool(name="big", bufs=2) as big,
            tc.tile_pool(name="ps", bufs=2, space="PSUM") as psp,
            tc.tile_pool(name="pst", bufs=2, space="PSUM") as pst,
            tc.tile_pool(name="gbuf", bufs=2) as gbuf,
            tc.tile_pool(name="vbuf", bufs=2) as vbuf,
            tc.tile_pool(name="obuf", bufs=2) as obuf,
        ):
            # ---- loads: conv1/idx path first, conv2/scale + main-conv later
            xpad = imgs.tile([C, 66 * 66], F16)
            xtpad = imgs.tile([C, 66 * 66], F16)
            wom = wts.tile([C, 9, 18], F16)
            wmt = wts.tile([C, 9, 9], F16)
            ob = wts.tile([18, 1], F32)
            mb = wts.tile([9, 1], F32)
            id16 = wts.tile([128, 128], F16)
            selt = wts.tile([128, 9, 3, 128], F16)
            basey = wts.tile([128, 9, KCH], F32)
            basex = wts.tile([128, 9, KCH], F32)
            w2 = wts.tile([C, 9, 2, 128], F16)
            nc.sync.dma_start(xpad[:, 0:2244], d["xpad"][:, 0:2244])
            nc.sync.dma_start(xpad[:, 2244:], d["xpad"][:, 2244:])
            nc.sync.dma_start(wom[:], d["wom"][:])
            nc.sync.dma_start(ob[:], d["ob"][:])
            nc.sync.dma_start(id16[:], d["id16"][:])
            nc.sync.dma_start(selt[:], d["sel"][:])
            nc.sync.dma_start(basey[:], d["basey"][:])
            nc.sync.dma_start(basex[:], d["basex"][:])
            nc.sync.dma_start(wmt[:], d["wmt"][:])
            nc.sync.dma_start(mb[:], d["mb"][:])
            nc.sync.dma_start(xtpad[:], d["xtpad"][:])
            nc.sync.dma_start(w2[:], d["w2"][:])

            # ---- PE p-state warmup on the identity while xpad streams in
            for _ in range(24):
                wpt = pst.tile([128, 128], F16, tag="tv", name="wpt")
                nc.tensor.transpose(wpt[:], id16[:], id16[:])

            # ---- conv1 (offsets, 18ch over xpad), transposes interleaved
            ocm = big.tile([128, PIX], F16, tag="big")
            opm = meta.tile([128, KCH, 18], F16)   # pi = 128k+p

            def conv1_half(hk):
                for tl in range(4 * hk, 4 * hk + 4):
                    po = psp.tile([18, 512], F32, tag="mm", name="po")
                    for t in range(9):
                        dy, dx = t // 3, t % 3
                        off = dy * 66 + dx + tl * 8 * 66
                        rhs1 = bass.AP(
                            tensor=xpad[:].tensor, offset=xpad[:].offset + off,
                            ap=[list(xpad[:].ap[0]), [66, 8], [1, 64]],
                        )
                        nc.tensor.matmul(po[:], wom[:, t, :], rhs1,
                                         start=(t == 0), stop=(t == 8))
                    nc.scalar.activation(ocm[0:18, tl * 512:(tl + 1) * 512],
                                         po[:],
                                         mybir.ActivationFunctionType.Identity,
                                         bias=ob[:], scale=1.0)
                    for k in range(4 * tl, 4 * tl + 4):
                        pt = pst.tile([128, 128], F16, tag="tr", name="pt")
                        nc.tensor.transpose(pt[:], ocm[:, k * 128:(k + 1) * 128],
                                            id16[:])
                        nc.scalar.copy(opm[:, k, :], pt[:, 0:18])

            # ---- per-n2: selection matmuls -> positions -> idx + fracs.
            # Split by k-half: half 0 covers sp=0's chunks, so its gathers
            # start after only half the pipeline latency.
            idxt = meta.tile([128, 9, KCH], I32)
            sb4 = meta.tile([128, 9, KCH, 4], F32)
            KH = KCH // 2

            def meta_half(hk):
                for n2 in range(9):
                    oyx = pst.tile([128, KH, 2], F32, tag="oyx", name="oyx")
                    for r in range(3):
                        a2 = n2 // 3
                        ch = 3 * r + a2
                        rhs = bass.AP(
                            tensor=opm[:].tensor,
                            offset=opm[:].offset + ch + hk * KH * 18,
                            ap=[list(opm[:].ap[0]), [18, KH], [9, 2]],
                        )
                        nc.tensor.matmul(oyx[:], selt[:, n2, r, :], rhs,
                                         start=(r == 0), stop=(r == 2))
                    kr = slice(hk * KH, (hk + 1) * KH)
                    P = meta.tile([128, KH, 2], F32, tag="P", name="P")
                    nc.vector.tensor_add(P[:, :, 0], oyx[:, :, 0],
                                         basey[:, n2, kr])
                    nc.vector.tensor_add(P[:, :, 1], oyx[:, :, 1],
                                         basex[:, n2, kr])
                    nc.vector.tensor_scalar_max(P[:], P[:], 0.0)
                    nc.vector.tensor_scalar_min(P[:], P[:], 63.0)
                    R0 = meta.tile([128, KH, 2], F32, tag="R0", name="R0")
                    nc.vector.tensor_scalar(R0[:], P[:], -0.5, 12582912.0,
                                            AO.add, AO.add)
                    nc.vector.tensor_scalar_add(R0[:], R0[:], -12582912.0)
                    Fh = meta.tile([128, KH, 2], F32, tag="Fh", name="Fh")
                    nc.vector.tensor_sub(Fh[:], P[:], R0[:])
                    f00 = meta.tile([128, KH], F32, tag="f00", name="f00")
                    nc.vector.scalar_tensor_tensor(
                        f00[:], R0[:, :, 1], 64.0, R0[:, :, 0], AO.mult, AO.add)
                    nc.vector.tensor_copy(idxt[:, n2, kr], f00[:])
                    # bilinear-only corner weights (modulation folded later):
                    # c0=(1-F1)(1-F0) c1=(1-F1)F0 c2=F1(1-F0) c3=F1*F0
                    nc.vector.tensor_scalar(sb4[:, n2, kr, 0], Fh[:, :, 1],
                                            -1.0, 1.0, AO.mult, AO.add)
                    nc.vector.tensor_mul(sb4[:, n2, kr, 1], sb4[:, n2, kr, 0],
                                         Fh[:, :, 0])
                    nc.vector.tensor_sub(sb4[:, n2, kr, 0], sb4[:, n2, kr, 0],
                                         sb4[:, n2, kr, 1])
                    nc.vector.tensor_mul(sb4[:, n2, kr, 3], Fh[:, :, 1],
                                         Fh[:, :, 0])
                    nc.vector.tensor_sub(sb4[:, n2, kr, 2], Fh[:, :, 1],
                                         sb4[:, n2, kr, 3])

            # ---- conv2 (modulation, 9ch over xtpad) + sigmoid, per-half
            mcm = big.tile([128, PIX], F16, tag="big")
            mpm = meta.tile([128, KCH, 9], F32)    # pi2' = 128k+p

            def conv2_half(hk):
                for tl in range(4 * hk, 4 * hk + 4):
                    pm = psp.tile([9, 512], F32, tag="mm", name="pm")
                    for t in range(9):
                        dy, dx = t // 3, t % 3
                        off = dy * 66 + dx + tl * 8 * 66
                        rhs2 = bass.AP(
                            tensor=xtpad[:].tensor,
                            offset=xtpad[:].offset + off,
                            ap=[list(xtpad[:].ap[0]), [66, 8], [1, 64]],
                        )
                        nc.tensor.matmul(pm[:], wmt[:, t, :], rhs2,
                                         start=(t == 0), stop=(t == 8))
                    nc.scalar.activation(mcm[0:9, tl * 512:(tl + 1) * 512],
                                         pm[:],
                                         mybir.ActivationFunctionType.Sigmoid,
                                         bias=mb[:], scale=1.0)
                    for k in range(4 * tl, 4 * tl + 4):
                        pt2 = pst.tile([128, 128], F16, tag="tr", name="pt2")
                        nc.tensor.transpose(pt2[:],
                                            mcm[:, k * 128:(k + 1) * 128],
                                            id16[:])
                        nc.scalar.copy(mpm[:, k, :], pt2[:, 0:9])

            # ---- scales: fold modulation into bilinear weights (tiny
            # broadcast muls; conv2-dependent work kept off the combine path)
            scal = meta.tile([128, 9, KCH, 4], F16)

            def scal_fold(hk):
                kr = slice(hk * KH, (hk + 1) * KH)
                for n2 in range(9):
                    mb4 = bass.AP(
                        tensor=mpm[:].tensor,
                        offset=mpm[:].offset + n2 + hk * KH * 9,
                        ap=[list(mpm[:].ap[0]), [9, KH], [0, 4]],
                    )
                    nc.vector.tensor_mul(scal[:, n2, kr], sb4[:, n2, kr], mb4)

            conv1_half(0)
            meta_half(0)
            conv2_half(0)
            scal_fold(0)
            conv1_half(1)
            meta_half(1)
            conv2_half(1)
            scal_fold(1)

            # ---- per spatial-half: gather + combine + transpose; then main conv
            for sp in range(2):
                vc = vbuf.tile([C, 9, 16 * 128], F16, tag="vc")
                for n2 in range(9):
                    g = gbuf.tile([128, 16, 128, 4], F16, tag="g")
                    for kk in range(16):
                        k = sp * 16 + kk
                        dst = bass.AP(
                            tensor=g[:].tensor,
                            offset=g[:].offset + kk * 512,
                            ap=[list(g[:].ap[0]), [1, 512]],
                        )
                        nc.gpsimd.indirect_dma_start(
                            out=dst, out_offset=None,
                            in_=d["ptab"][:],
                            in_offset=bass.IndirectOffsetOnAxis(
                                ap=idxt[:, n2, k:k + 1], axis=0),
                        )
                    # per-half combine chains (all packed fp16 at 2x except
                    # the strided final add); h0 completes while h1 gathers
                    tmp = big.tile([128, 16, 128, 2], F16, tag="big")
                    va = gbuf.tile([128, 16, 128], F16, tag="va")
                    for h in range(2):
                        gm = bass.AP(
                            tensor=g[:].tensor,
                            offset=g[:].offset + h * 8 * 512,
                            ap=[list(g[:].ap[0]), [512, 8], [4, 128], [1, 4]],
                        )
                        sc = bass.AP(
                            tensor=scal[:].tensor,
                            offset=scal[:].offset + n2 * (KCH * 4)
                            + (sp * 16 + h * 8) * 4,
                            ap=[list(scal[:].ap[0]), [4, 8], [0, 128], [1, 4]],
                        )
                        nc.vector.tensor_mul(gm, gm, sc)
                        a0 = bass.AP(
                            tensor=g[:].tensor,
                            offset=g[:].offset + h * 8 * 512,
                            ap=[list(g[:].ap[0]), [512, 8], [4, 128], [1, 2]],
                        )
                        a1 = bass.AP(
                            tensor=g[:].tensor,
                            offset=g[:].offset + h * 8 * 512 + 2,
                            ap=[list(g[:].ap[0]), [512, 8], [4, 128], [1, 2]],
                        )
                        to = bass.AP(
                            tensor=tmp[:].tensor,
                            offset=tmp[:].offset + h * 8 * 256,
                            ap=[list(tmp[:].ap[0]), [256, 8], [2, 128], [1, 2]],
                        )
                        nc.vector.tensor_add(to, a0, a1)
                        t0 = bass.AP(
                            tensor=tmp[:].tensor,
                            offset=tmp[:].offset + h * 8 * 256,
                            ap=[list(tmp[:].ap[0]), [256, 8], [2, 128]],
                        )
                        t1 = bass.AP(
                            tensor=tmp[:].tensor,
                            offset=tmp[:].offset + h * 8 * 256 + 1,
                            ap=[list(tmp[:].ap[0]), [256, 8], [2, 128]],
                        )
                        nc.vector.tensor_add(va[:, h * 8:(h + 1) * 8, :], t0, t1)
                        for kk in range(h * 8, h * 8 + 8):
                            ptv = pst.tile([128, 128], F16, tag="tv")
                            nc.tensor.transpose(ptv[:], va[:, kk, :], id16[:])
                            nc.scalar.copy(vc[:, n2, kk * 128:(kk + 1) * 128],
                                           ptv[:])

                # main conv on this spatial half (pi2' in [sp*2048, +2048))
                outsbs = []
                for hf in range(2):
                    outsbs.append(obuf.tile([128, 16 * 128], F16, tag=f"osb{hf}",
                                            name=f"outsb{hf}"))
                for tl in range(4):
                    for hf in range(2):
                        outsb = outsbs[hf]
                        acc = psp.tile([128, 512], F32, tag="mm")
                        for n2 in range(9):
                            nc.tensor.matmul(
                                acc[:], w2[:, n2, hf, :],
                                vc[:, n2, tl * 512:(tl + 1) * 512],
                                start=(n2 == 0), stop=(n2 == 8))
                        # acc covers pi2' = sp*2048 + tl*512 + [0,512):
                        # j2 = (pi2'//64), i2 = pi2'%64 -> dst elem i2*32 + (j2 - 32*sp)
                        dstap = bass.AP(
                            tensor=outsb[:].tensor,
                            offset=outsb[:].offset + 8 * tl,
                            ap=[list(outsb[:].ap[0]), [1, 8], [32, 64]],
                        )
                        nc.scalar.copy(dstap, acc[:])
                # DMA: out[128hf + o, i2, 32sp + j2'] <- outsb[o, i2*32 + j2']
                for hf in range(2):
                    dd = d["out"]
                    dram = bass.AP(
                        tensor=dd[:].tensor,
                        offset=dd[:].offset + hf * 128 * PIX + 32 * sp,
                        ap=[[PIX, 128], [64, 64], [1, 32]],
                    )
                    nc.sync.dma_start(dram, outsbs[hf][:])

    nc.compile()
    _CACHE["nc"] = nc
    return nc


def _host_inputs(b_x, offset_w, offset_b, mod_w, mod_b, conv_w):
    hc = _build_host_constants()
    img = b_x.astype(np.float32)
    imgT = np.ascontiguousarray(img.transpose(0, 2, 1))
    wom = np.zeros((9, C, 18), np.float16)
    wmt = np.zeros((9, C, 9), np.float16)
    for t in range(9):
        dy, dx = t // 3, t % 3
        wom[t] = offset_w[:, :, dy, dx].T
        wmt[3 * dx + dy] = mod_w[:, :, dy, dx].T
    wom = np.ascontiguousarray(wom.transpose(1, 0, 2)).reshape(C, 9 * 18)
    wmt = np.ascontiguousarray(wmt.transpose(1, 0, 2)).reshape(C, 9 * 9)
    w2 = np.zeros((9, 2, C, 128), np.float16)
    for n2 in range(9):
        a2, e2 = n2 // 3, n2 % 3
        for hf in range(2):
            w2[n2, hf] = conv_w[128 * hf:128 * (hf + 1), :, a2, e2].T.astype(
                np.float16)
    w2 = np.ascontiguousarray(w2.transpose(2, 0, 1, 3)).reshape(C, 9 * 2 * 128)
    return {
        "xpad": _pad66(img),
        "xtpad": _pad66(imgT),
        "ptab": _patch_table(img),
        "wom": wom,
        "wmt": wmt,
        "ob": offset_b.reshape(18, 1).astype(np.float32),
        "mb": mod_b.reshape(9, 1).astype(np.float32),
        "sel": hc["sel"],
        "basey": hc["basey"],
        "basex": hc["basex"],
        "w2": w2,
        "id16": hc["ident16"],
    }


def kernel(x, offset_w, offset_b, mod_w, mod_b, conv_w):
    nc = _build_program()
    in_maps = [
        _host_inputs(x[b], offset_w, offset_b, mod_w, mod_b, conv_w)
        for b in range(B)
    ]
    res = run_bass_kernel_spmd(nc, in_maps, core_ids=list(range(B)))
    out = np.stack([res.results[b]["out"].reshape(OUT, H, W) for b in range(B)])
    return out.astype(np.float32)


if __name__ == "__main__":
    rng = np.random.default_rng(0)
    ins = {
        "x": rng.standard_normal((B, C, H, W), dtype=np.float32),
        "offset_w": (rng.standard_normal((18, C, 3, 3)) / 34).astype(np.float32),
        "offset_b": (rng.standard_normal(18) * 0.01).astype(np.float32),
        "mod_w": (rng.standard_normal((9, C, 3, 3)) / 34).astype(np.float32),
        "mod_b": (rng.standard_normal(9) * 0.01).astype(np.float32),
        "conv_w": (rng.standard_normal((OUT, C, 3, 3)) / 34).astype(np.float32),
    }
    o = kernel(**ins)
    print("out", o.shape, o.dtype, np.abs(o).max())


# revision 30
# speedup vs baseline: 1.0564x; 1.0233x over previous
# Deformable-conv (DCNv2-style, scrambled-reshape variant) Trainium2 Bass kernel.
# Data-parallel over batch: 8 samples -> 8 NeuronCores.
#
# Per-core pipeline (all layouts derived + validated against the reference in numpy):
#   1. offset conv (18ch, fp16) over padded x -> PE-transpose -> per-n2 selection
#      matmuls -> flat 2x2-patch index f00 + bilinear fracs; gathers can start as
#      soon as idxt[n2] lands. Modulation conv (9ch) over padded x^T + scale
#      table build run behind the first gathers.
#   2. 16 indirect-DMA gathers per (sp, n2) from a host-built patch table
#      (row f = [128 ch x 4 corners] of flat pixels [f, f+1, f+64, f+65], fp16,
#      corner-minor so the scale multiply runs at 2x DVE rate).
#   3. Combine: 2 half-tile muls by (modulation x bilinear) scales, 2 half-tile
#      corner-pair adds (all packed fp16, 2x DVE), one strided final add.
#   4. PE-transpose back to channel-major, Act drains PSUM into vc.
#   5. Main conv = 9 accumulated fp16 matmuls per output tile; Act PSUM copies
#      write through a transposed AP to undo the pi2' ordering.
import sys

import numpy as np

sys.path.insert(0, "/opt/trn_rl_repo")

import concourse.bass as bass
import concourse.bacc as bacc
import concourse.mybir as mybir
from concourse import tile
from concourse.bass_utils import run_bass_kernel_spmd

F32 = mybir.dt.float32
F16 = mybir.dt.float16
I32 = mybir.dt.int32

B, C, H, W = 8, 128, 64, 64
OUT = 256
PIX = H * W            # 4096
KCH = 32               # pixel-major chunks (4096 / 128)
TROWS = 4224           # patch table rows (4096 + pad for f+65 reads)

_CACHE = {}


def _build_host_constants():
    if "sel" in _CACHE:
        return _CACHE
    p2 = np.arange(128)
    k2 = np.arange(KCH)
    sel = np.zeros((9, 3, 128, 128), np.float16)   # [n2, r, p_src, p2]
    basey = np.zeros((9, 128, KCH), np.float32)
    basex = np.zeros((9, 128, KCH), np.float32)
    for n2 in range(9):
        a2, e2 = n2 // 3, n2 % 3
        i2 = p2 % 64
        r = (i2 + e2) % 3
        n = 3 * r + a2                       # source kernel point per partition
        J = (64 * e2 + i2) // 3              # source col j per partition
        c_src = 64 * (p2 // 64) + J          # source partition in pixel-major
        for rr in range(3):
            m = r == rr
            sel[n2, rr, c_src[m], p2[m]] = 1.0
        a = n // 3
        e = n % 3
        # y_u = i + a + o_y ; i = j2 = 2*k2 + p2//64
        basey[n2] = (2 * k2[None, :] + (p2 // 64)[:, None]) + a[:, None]
        basex[n2] = (J + e)[:, None] * np.ones((1, KCH), np.float32)
    _CACHE["sel"] = np.ascontiguousarray(
        sel.transpose(2, 0, 1, 3)).reshape(128, 9 * 3 * 128)
    _CACHE["basey"] = np.ascontiguousarray(
        basey.transpose(1, 0, 2)).reshape(128, 9 * KCH)
    _CACHE["basex"] = np.ascontiguousarray(
        basex.transpose(1, 0, 2)).reshape(128, 9 * KCH)
    _CACHE["ident16"] = np.eye(128, dtype=np.float16)
    return _CACHE


def _pad66(img):  # [C,64,64] -> [C, 66*66] zero-padded
    p = np.zeros((C, 66, 66), np.float16)
    p[:, 1:65, 1:65] = img
    return p.reshape(C, 66 * 66)


def _patch_table(img):  # [C,64,64] f32 -> [TROWS, 512] fp16, rows [ch, corner]
    flat = np.zeros((C, TROWS + 65), np.float16)
    flat[:, :PIX] = img.reshape(C, PIX).astype(np.float16)
    f = np.arange(TROWS)
    tab = np.stack(
        [flat[:, f], flat[:, f + 1], flat[:, f + 64], flat[:, f + 65]], axis=2
    )  # [C, TROWS, 4]
    return np.ascontiguousarray(tab.transpose(1, 0, 2)).reshape(TROWS, 512)


def _build_program():
    if "nc" in _CACHE:
        return _CACHE["nc"]
    nc = bacc.Bacc()
    d = {}
    d["xpad"] = nc.dram_tensor("xpad", [C, 66 * 66], F16, kind="ExternalInput")
    d["xtpad"] = nc.dram_tensor("xtpad", [C, 66 * 66], F16, kind="ExternalInput")
    d["ptab"] = nc.dram_tensor("ptab", [TROWS, 512], F16, kind="ExternalInput")
    d["wom"] = nc.dram_tensor("wom", [C, 9 * 18], F16, kind="ExternalInput")
    d["wmt"] = nc.dram_tensor("wmt", [C, 9 * 9], F16, kind="ExternalInput")
    d["ob"] = nc.dram_tensor("ob", [18, 1], F32, kind="ExternalInput")
    d["mb"] = nc.dram_tensor("mb", [9, 1], F32, kind="ExternalInput")
    d["sel"] = nc.dram_tensor("sel", [128, 9 * 3 * 128], F16, kind="ExternalInput")
    d["basey"] = nc.dram_tensor("basey", [128, 9 * KCH], F32, kind="ExternalInput")
    d["basex"] = nc.dram_tensor("basex", [128, 9 * KCH], F32, kind="ExternalInput")
    d["w2"] = nc.dram_tensor("w2", [C, 9 * 2 * 128], F16, kind="ExternalInput")
    d["id16"] = nc.dram_tensor("id16", [128, 128], F16, kind="ExternalInput")
    d["out"] = nc.dram_tensor("out", [OUT, PIX], F16, kind="ExternalOutput")

    AO = mybir.AluOpType

    with tile.TileContext(nc) as tc:
        with (
            tc.tile_pool(name="imgs", bufs=1) as imgs,
            tc.tile_pool(name="wts", bufs=1) as wts,
            tc.tile_pool(name="meta", bufs=1) as meta,
            tc.tile_pool(name="big", bufs=2) as big,
            tc.tile_pool(name="ps", bufs=2, space="PSUM") as psp,
            tc.tile_pool(name="pst", bufs=2, space="PSUM") as pst,
            tc.tile_pool(name="gbuf", bufs=2) as gbuf,
            tc.tile_pool(name="vbuf", bufs=2) as vbuf,
            tc.tile_pool(name="obuf", bufs=2) as obuf,
        ):
            # ---- loads: conv1/idx path first, conv2/scale + main-conv later
            xpad = imgs.tile([C, 66 * 66], F16)
            xtpad = imgs.tile([C, 66 * 66], F16)
            wom = wts.tile([C, 9, 18], F16)
            wmt = wts.tile([C, 9, 9], F16)
            ob = wts.tile([18, 1], F32)
            mb = wts.tile([9, 1], F32)
            id16 = wts.tile([128, 128], F16)
            selt = wts.tile([128, 9, 3, 128], F16)
            basey = wts.tile([128, 9, KCH], F32)
            basex = wts.tile([128, 9, KCH], F32)
            w2 = wts.tile([C, 9, 2, 128], F16)
            nc.sync.dma_start(xpad[:, 0:2244], d["xpad"][:, 0:2244])
            nc.sync.dma_start(xpad[:, 2244:], d["xpad"][:, 2244:])
            nc.sync.dma_start(wom[:], d["wom"][:])
            nc.sync.dma_start(ob[:], d["ob"][:])
            nc.sync.dma_start(id16[:], d["id16"][:])
            nc.sync.dma_start(selt[:], d["sel"][:])
            nc.sync.dma_start(basey[:], d["basey"][:])
            nc.sync.dma_start(basex[:], d["basex"][:])
            nc.sync.dma_start(wmt[:], d["wmt"][:])
            nc.sync.dma_start(mb[:], d["mb"][:])
            nc.sync.dma_start(xtpad[:], d["xtpad"][:])
            nc.sync.dma_start(w2[:], d["w2"][:])

            # ---- PE p-state warmup on the identity while xpad streams in
            for _ in range(24):
                wpt = pst.tile([128, 128], F16, tag="tv", name="wpt")
                nc.tensor.transpose(wpt[:], id16[:], id16[:])

            # ---- conv1 (offsets, 18ch over xpad), transposes interleaved
            ocm = big.tile([128, PIX], F16, tag="big")
            opm = meta.tile([128, KCH, 18], F16)   # pi = 128k+p

            def conv1_half(hk):
                for tl in range(4 * hk, 4 * hk + 4):
                    po = psp.tile([18, 512], F32, tag="mm", name="po")
                    for t in range(9):
                        dy, dx = t // 3, t % 3
                        off = dy * 66 + dx + tl * 8 * 66
                        rhs1 = bass.AP(
                            tensor=xpad[:].tensor, offset=xpad[:].offset + off,
                            ap=[list(xpad[:].ap[0]), [66, 8], [1, 64]],
                        )
                        nc.tensor.matmul(po[:], wom[:, t, :], rhs1,
                                         start=(t == 0), stop=(t == 8))
                    nc.scalar.activation(ocm[0:18, tl * 512:(tl + 1) * 512],
                                         po[:],
                                         mybir.ActivationFunctionType.Identity,
                                         bias=ob[:], scale=1.0)
                    for k in range(4 * tl, 4 * tl + 4):
                        pt = pst.tile([128, 128], F16, tag="tr", name="pt")
                        nc.tensor.transpose(pt[:], ocm[:, k * 128:(k + 1) * 128],
                                            id16[:])
                        nc.scalar.copy(opm[:, k, :], pt[:, 0:18])

            # ---- per-n2: selection matmuls -> positions -> idx + fracs.
            # Split by k-half: half 0 covers sp=0's chunks, so its gathers
            # start after only half the pipeline latency.
            idxt = meta.tile([128, 9, KCH], I32)
            sb4 = meta.tile([128, 9, KCH, 4], F32)
            KH = KCH // 2

            def meta_half(hk):
                for n2 in range(9):
                    oyx = pst.tile([128, KH, 2], F32, tag="oyx", name="oyx")
                    for r in range(3):
                        a2 = n2 // 3
                        ch = 3 * r + a2
                        rhs = bass.AP(
                            tensor=opm[:].tensor,
                            offset=opm[:].offset + ch + hk * KH * 18,
                            ap=[list(opm[:].ap[0]), [18, KH], [9, 2]],
                        )
                        nc.tensor.matmul(oyx[:], selt[:, n2, r, :], rhs,
                                         start=(r == 0), stop=(r == 2))
                    kr = slice(hk * KH, (hk + 1) * KH)
                    P = meta.tile([128, KH, 2], F32, tag="P", name="P")
                    nc.vector.tensor_add(P[:, :, 0], oyx[:, :, 0],
                                         basey[:, n2, kr])
                    nc.vector.tensor_add(P[:, :, 1], oyx[:, :, 1],
                                         basex[:, n2, kr])
                    nc.vector.tensor_scalar_max(P[:], P[:], 0.0)
                    nc.vector.tensor_scalar_min(P[:], P[:], 63.0)
                    R0 = meta.tile([128, KH, 2], F32, tag="R0", name="R0")
                    nc.vector.tensor_scalar(R0[:], P[:], -0.5, 12582912.0,
                                            AO.add, AO.add)
                    nc.vector.tensor_scalar_add(R0[:], R0[:], -12582912.0)
                    Fh = meta.tile([128, KH, 2], F32, tag="Fh", name="Fh")
                    nc.vector.tensor_sub(Fh[:], P[:], R0[:])
                    f00 = meta.tile([128, KH], F32, tag="f00", name="f00")
                    nc.vector.scalar_tensor_tensor(
                        f00[:], R0[:, :, 1], 64.0, R0[:, :, 0], AO.mult, AO.add)
                    nc.vector.tensor_copy(idxt[:, n2, kr], f00[:])
                    # bilinear-only corner weights (modulation folded later):
                    # c0=(1-F1)(1-F0) c1=(1-F1)F0 c2=F1(1-F0) c3=F1*F0
                    nc.vector.tensor_scalar(sb4[:, n2, kr, 0], Fh[:, :, 1],
                                            -1.0, 1.0, AO.mult, AO.add)
                    nc.vector.tensor_mul(sb4[:, n2, kr, 1], sb4[:, n2, kr, 0],
                                         Fh[:, :, 0])
                    nc.vector.tensor_sub(sb4[:, n2, kr, 0], sb4[:, n2, kr, 0],
                                         sb4[:, n2, kr, 1])
                    nc.vector.tensor_mul(sb4[:, n2, kr, 3], Fh[:, :, 1],
                                         Fh[:, :, 0])
                    nc.vector.tensor_sub(sb4[:, n2, kr, 2], Fh[:, :, 1],
                                         sb4[:, n2, kr, 3])

            # ---- conv2 (modulation, 9ch over xtpad) + sigmoid, per-half
            mcm = big.tile([128, PIX], F16, tag="big")
            mpm = meta.tile([128, KCH, 9], F32)    # pi2' = 128k+p

            def conv2_half(hk):
                for tl in range(4 * hk, 4 * hk + 4):
                    pm = psp.tile([9, 512], F32, tag="mm", name="pm")
                    for t in range(9):
                        dy, dx = t // 3, t % 3
                        off = dy * 66 + dx + tl * 8 * 66
                        rhs2 = bass.AP(
                            tensor=xtpad[:].tensor,
                            offset=xtpad[:].offset + off,
                            ap=[list(xtpad[:].ap[0]), [66, 8], [1, 64]],
                        )
                        nc.tensor.matmul(pm[:], wmt[:, t, :], rhs2,
                                         start=(t == 0), stop=(t == 8))
                    nc.scalar.activation(mcm[0:9, tl * 512:(tl + 1) * 512],
                                         pm[:],
                                         mybir.ActivationFunctionType.Sigmoid,
                                         bias=mb[:], scale=1.0)
                    for k in range(4 * tl, 4 * tl + 4):
                        pt2 = pst.tile([128, 128], F16, tag="tr", name="pt2")
                        nc.tensor.transpose(pt2[:],
                                            mcm[:, k * 128:(k + 1) * 128],
                                            id16[:])
                        nc.scalar.copy(mpm[:, k, :], pt2[:, 0:9])

            # ---- scales: fold modulation into bilinear weights (tiny
            # broadcast muls; conv2-dependent work kept off the combine path)
            scal = meta.tile([128, 9, KCH, 4], F16)

            def scal_fold(hk):
                kr = slice(hk * KH, (hk + 1) * KH)
                for n2 in range(9):
                    mb4 = bass.AP(
                        tensor=mpm[:].tensor,
                        offset=mpm[:].offset + n2 + hk * KH * 9,
                        ap=[list(mpm[:].ap[0]), [9, KH], [0, 4]],
                    )
                    nc.vector.tensor_mul(scal[:, n2, kr], sb4[:, n2, kr], mb4)

            conv1_half(0)
            meta_half(0)
            conv2_half(0)
            scal_fold(0)
            conv1_half(1)
            meta_half(1)
            conv2_half(1)
            scal_fold(1)

            # ---- per spatial-half: gather + combine + transpose; then main conv
            for sp in range(2):
                vc = vbuf.tile([C, 9, 16 * 128], F16, tag="vc")
                paccs = [
                    obuf.tile([128, 16 * 128], F16, tag=f"pacc{hf}", bufs=1,
                              name=f"pacc{hf}")
                    for hf in range(2)
                ]
                for n2 in range(9):
                    g = gbuf.tile([128, 16, 128, 4], F16, tag="g")
                    for kk in range(16):
                        k = sp * 16 + kk
                        dst = bass.AP(
                            tensor=g[:].tensor,
                            offset=g[:].offset + kk * 512,
                            ap=[list(g[:].ap[0]), [1, 512]],
                        )
                        nc.gpsimd.indirect_dma_start(
                            out=dst, out_offset=None,
                            in_=d["ptab"][:],
                            in_offset=bass.IndirectOffsetOnAxis(
                                ap=idxt[:, n2, k:k + 1], axis=0),
                        )
                    # per-half combine chains (all packed fp16 at 2x except
                    # the strided final add); h0 completes while h1 gathers
                    tmp = big.tile([128, 16, 128, 2], F16, tag="big")
                    va = gbuf.tile([128, 16, 128], F16, tag="va")
                    for h in range(2):
                        gm = bass.AP(
                            tensor=g[:].tensor,
                            offset=g[:].offset + h * 8 * 512,
                            ap=[list(g[:].ap[0]), [512, 8], [4, 128], [1, 4]],
                        )
                        sc = bass.AP(
                            tensor=scal[:].tensor,
                            offset=scal[:].offset + n2 * (KCH * 4)
                            + (sp * 16 + h * 8) * 4,
                            ap=[list(scal[:].ap[0]), [4, 8], [0, 128], [1, 4]],
                        )
                        nc.vector.tensor_mul(gm, gm, sc)
                        a0 = bass.AP(
                            tensor=g[:].tensor,
                            offset=g[:].offset + h * 8 * 512,
                            ap=[list(g[:].ap[0]), [512, 8], [4, 128], [1, 2]],
                        )
                        a1 = bass.AP(
                            tensor=g[:].tensor,
                            offset=g[:].offset + h * 8 * 512 + 2,
                            ap=[list(g[:].ap[0]), [512, 8], [4, 128], [1, 2]],
                        )
                        to = bass.AP(
                            tensor=tmp[:].tensor,
                            offset=tmp[:].offset + h * 8 * 256,
                            ap=[list(tmp[:].ap[0]), [256, 8], [2, 128], [1, 2]],
                        )
                        nc.vector.tensor_add(to, a0, a1)
                        t0 = bass.AP(
                            tensor=tmp[:].tensor,
                            offset=tmp[:].offset + h * 8 * 256,
                            ap=[list(tmp[:].ap[0]), [256, 8], [2, 128]],
                        )
                        t1 = bass.AP(
                            tensor=tmp[:].tensor,
                            offset=tmp[:].offset + h * 8 * 256 + 1,
                            ap=[list(tmp[:].ap[0]), [256, 8], [2, 128]],
                        )
                        nc.vector.tensor_add(va[:, h * 8:(h + 1) * 8, :], t0, t1)
                        for kk in range(h * 8, h * 8 + 8):
                            ptv = pst.tile([128, 128], F16, tag="tv")
                            nc.tensor.transpose(ptv[:], va[:, kk, :], id16[:])
                            nc.scalar.copy(vc[:, n2, kk * 128:(kk + 1) * 128],
                                           ptv[:])

                    if n2 == 6:
                        # partial main conv over n2=0..6 while last gathers run
                        for hf in range(2):
                            for tl in range(4):
                                acc = psp.tile([128, 512], F32, tag="mm")
                                for j in range(7):
                                    nc.tensor.matmul(
                                        acc[:], w2[:, j, hf, :],
                                        vc[:, j, tl * 512:(tl + 1) * 512],
                                        start=(j == 0), stop=(j == 6))
                                nc.scalar.copy(
                                    paccs[hf][:, tl * 512:(tl + 1) * 512],
                                    acc[:])
                # main conv: n2<=6 was accumulated mid-stream into pacc;
                # tail does pacc-reload (identity matmul) + n2=7,8 only.
                for hf in range(2):
                    outsb = obuf.tile([128, 16 * 128], F16, tag=f"osb{hf}",
                                      bufs=1, name=f"outsb{hf}")
                    for tl in range(4):
                        acc = psp.tile([128, 512], F32, tag="mm")
                        nc.tensor.matmul(acc[:], id16[:],
                                         paccs[hf][:, tl * 512:(tl + 1) * 512],
                                         start=True, stop=False)
                        for n2 in (7, 8):
                            nc.tensor.matmul(
                                acc[:], w2[:, n2, hf, :],
                                vc[:, n2, tl * 512:(tl + 1) * 512],
                                start=False, stop=(n2 == 8))
                        nc.scalar.copy(outsb[:, tl * 512:(tl + 1) * 512], acc[:])
                    # contiguous store in pi2' order; host un-permutes
                    nc.sync.dma_start(
                        d["out"][128 * hf:128 * (hf + 1),
                                 2048 * sp:2048 * (sp + 1)],
                        outsb[:])

    nc.compile()
    _CACHE["nc"] = nc
    return nc


def _host_inputs(b_x, offset_w, offset_b, mod_w, mod_b, conv_w):
    hc = _build_host_constants()
    img = b_x.astype(np.float32)
    imgT = np.ascontiguousarray(img.transpose(0, 2, 1))
    wom = np.zeros((9, C, 18), np.float16)
    wmt = np.zeros((9, C, 9), np.float16)
    for t in range(9):
        dy, dx = t // 3, t % 3
        wom[t] = offset_w[:, :, dy, dx].T
        wmt[3 * dx + dy] = mod_w[:, :, dy, dx].T
    wom = np.ascontiguousarray(wom.transpose(1, 0, 2)).reshape(C, 9 * 18)
    wmt = np.ascontiguousarray(wmt.transpose(1, 0, 2)).reshape(C, 9 * 9)
    w2 = np.zeros((9, 2, C, 128), np.float16)
    for n2 in range(9):
        a2, e2 = n2 // 3, n2 % 3
        for hf in range(2):
            w2[n2, hf] = conv_w[128 * hf:128 * (hf + 1), :, a2, e2].T.astype(
                np.float16)
    w2 = np.ascontiguousarray(w2.transpose(2, 0, 1, 3)).reshape(C, 9 * 2 * 128)
    return {
        "xpad": _pad66(img),
        "xtpad": _pad66(imgT),
        "ptab": _patch_table(img),
        "wom": wom,
        "wmt": wmt,
        "ob": offset_b.reshape(18, 1).astype(np.float32),
        "mb": mod_b.reshape(9, 1).astype(np.float32),
        "sel": hc["sel"],
        "basey": hc["basey"],
        "basex": hc["basex"],
        "w2": w2,
        "id16": hc["ident16"],
    }


def kernel(x, offset_w, offset_b, mod_w, mod_b, conv_w):
    nc = _build_program()
    in_maps = [
        _host_inputs(x[b], offset_w, offset_b, mod_w, mod_b, conv_w)
        for b in range(B)
    ]
    res = run_bass_kernel_spmd(nc, in_maps, core_ids=list(range(B)))
    out = np.stack([
        res.results[b]["out"].reshape(OUT, W, H).transpose(0, 2, 1)
        for b in range(B)
    ])
    return out.astype(np.float32)


if __name__ == "__main__":
    rng = np.random.default_rng(0)
    ins = {
        "x": rng.standard_normal((B, C, H, W), dtype=np.float32),
        "offset_w": (rng.standard_normal((18, C, 3, 3)) / 34).astype(np.float32),
        "offset_b": (rng.standard_normal(18) * 0.01).astype(np.float32),
        "mod_w": (rng.standard_normal((9, C, 3, 3)) / 34).astype(np.float32),
        "mod_b": (rng.standard_normal(9) * 0.01).astype(np.float32),
        "conv_w": (rng.standard_normal((OUT, C, 3, 3)) / 34).astype(np.float32),
    }
    o = kernel(**ins)
    print("out", o.shape, o.dtype, np.abs(o).max())
